# revision 41
# baseline (speedup 1.0000x reference)
"""DGCNN encoder Bass kernel for Trainium2, data-parallel over batch on 8 cores.

Per core (one sample, x: (2048, 3)):
  4 EdgeConv layers + final 1x1 conv + global max/avg pool -> (2048,) output row.

Key algebraic restructuring (exact, since the BN scale gamma*rsqrt(var+eps) > 0
and leaky-relu is monotone):
  edgeconv(x)[n] = bnlrelu( max_{j in knn(n)} (Wa @ x_j)  +  (Wb - Wa) @ x_n )
with W = [Wa | Wb] acting on [x_j - x_n ; x_n].  This removes the k=20 factor
from all matmuls; only the top-20 selection and a row-gather + max remain.

kNN selection per 128-row block: score[n, j] = 2 x_n.x_j - |x_j|^2 (row-shifted
negated squared distance, same per-row order) via PE matmuls, then three
max8 / max_index / match_replace rounds on DVE for the exact top-20 set,
operating directly on the PSUM score tile (no SBUF copy).

Neighbor gather: indices are bounced through DRAM into the 16-partition-wrapped
layout the GPSIMD indirect_copy expects (same index list for every partition
group), gathering u^T = (Wa @ x)^T rows; max over k=20 via a GPSIMD
tensor_reduce (keeps the DVE free for the top-k passes).

Engine placement: all simple DMAs ride the HWDGE path (nc.sync / SP engine)
so the GPSIMD Pool engine only runs the indirect gathers + reduces; BN+lrelu
is a single Act-engine Prelu activation (alpha=0.2).

Static layout transforms (x -> x^T, W -> Wa^T / (Wb-Wa)^T, W5^T chunks,
bn -> bn^T) happen host-side in numpy: element-granularity strided DRAM DMAs
abort the NRT on this stack, and contiguous feeds make them unnecessary.
"""
import sys
sys.path.insert(0, '/opt/trn_rl_repo')

import numpy as np
import concourse.bass as bass
import concourse.bacc as bacc
import concourse.tile as tile
from concourse import mybir

f32 = mybir.dt.float32
bf16 = mybir.dt.bfloat16
u16 = mybir.dt.uint16
Alu = mybir.AluOpType
Act = mybir.ActivationFunctionType

N = 2048
NBLK = N // 128
KNN = 20
NEG_SLOPE = 0.2
BN_EPS = 1e-5
NEG_BIG = -1e30

# (C_in, O) per edge-conv layer
LAYERS = [(3, 64), (64, 64), (64, 128), (128, 256)]
# W5^T host-side chunks aligned to the xcat source tiles
W5_CHUNKS = [(0, 64), (64, 128), (128, 256), (256, 384), (384, 512)]


def _ceil(a, b):
    return (a + b - 1) // b


def build_dgcnn(nc):
    """Emit the full per-core DGCNN program into nc."""
    xT_d = nc.dram_tensor("xT", [3, N], f32, kind="ExternalInput")
    WaT_d = [nc.dram_tensor(f"WaT{l+1}", [c, o], f32, kind="ExternalInput")
             for l, (c, o) in enumerate(LAYERS)]
    WdT_d = [nc.dram_tensor(f"WdT{l+1}", [c, o], f32, kind="ExternalInput")
             for l, (c, o) in enumerate(LAYERS)]
    W5T_d = [nc.dram_tensor(f"W5T_{lo}", [hi - lo, 1024], f32,
                            kind="ExternalInput") for lo, hi in W5_CHUNKS]
    bnT_d = [nc.dram_tensor(f"bnT{l+1}", [o, 4], f32, kind="ExternalInput")
             for l, (c, o) in enumerate(LAYERS)]
    bn5T_d = nc.dram_tensor("bn5T", [1024, 4], f32, kind="ExternalInput")
    out_d = nc.dram_tensor("out", [2048], f32, kind="ExternalOutput")
    # DRAM bounce for the index wrap-relayout, n-major: list[n*20+k] = idx[n,k]
    list_d = nc.dram_tensor("idxlist", [NBLK * 128 * KNN], u16, kind="Internal")

    with tile.TileContext(nc) as tc:
        from contextlib import ExitStack
        ctx = ExitStack()
        with ctx:
            persist = ctx.enter_context(tc.tile_pool(name="persist", bufs=1))
            work = ctx.enter_context(tc.tile_pool(name="work", bufs=2))

            onesC = persist.tile([128, 1], f32, tag="onesC")
            nc.vector.memset(onesC, 1.0)
            ones1 = persist.tile([1, 128], f32, tag="ones1")
            nc.vector.memset(ones1, 1.0)
            eps_t = persist.tile([128, 1], f32, tag="eps")
            nc.vector.memset(eps_t, BN_EPS)

            # ---- bn param prep: (C, 4) rows [gamma, beta, mean, var] ->
            #      scale (C,1), bias (C,1) tiles per 128-channel chunk
            def prep_bn(bn_dram, channels, name):
                scales, biases = [], []
                for t in range(_ceil(channels, 128)):
                    p = min(128, channels - t * 128)
                    raw = work.tile([128, 4], f32, tag="bnraw")
                    src = bass.AP(tensor=bn_dram, offset=t * 128 * 4,
                                  ap=[[4, p], [1, 4]])
                    nc.sync.dma_start(out=raw[:p, :], in_=src)
                    s_t = persist.tile([128, 1], f32, tag=f"{name}_s{t}")
                    b_t = persist.tile([128, 1], f32, tag=f"{name}_b{t}")
                    tmp = work.tile([128, 1], f32, tag="bntmp")
                    nc.scalar.activation(tmp[:p], raw[:p, 3:4], Act.Sqrt,
                                         bias=eps_t[:p], scale=1.0)
                    nc.vector.reciprocal(tmp[:p], tmp[:p])
                    nc.vector.tensor_mul(s_t[:p], raw[:p, 0:1], tmp[:p])
                    nc.vector.tensor_mul(tmp[:p], raw[:p, 2:3], s_t[:p])
                    nc.vector.tensor_sub(b_t[:p], raw[:p, 1:2], tmp[:p])
                    scales.append(s_t)
                    biases.append(b_t)
                return scales, biases

            bn_sb = [prep_bn(bnT_d[l], o, f"bn{l}")
                     for l, (c, o) in enumerate(LAYERS)]
            bn5_s, bn5_b = prep_bn(bn5T_d, 1024, "bn5")

            # ---- weights (already transposed host-side)
            WaT, WdT = [], []
            for l, (c, o) in enumerate(LAYERS):
                wa = persist.tile([max(c, 16), o], f32, tag=f"WaT{l}",
                                  name=f"WaT{l}")
                nc.sync.dma_start(out=wa[:c, :], in_=WaT_d[l][:, :])
                wd = persist.tile([max(c, 16), o], f32, tag=f"WdT{l}",
                                  name=f"WdT{l}")
                nc.sync.dma_start(out=wd[:c, :], in_=WdT_d[l][:, :])
                WaT.append(wa)
                WdT.append(wd)
            W5T = []
            for i, (lo, hi) in enumerate(W5_CHUNKS):
                # gpsimd (SWDGE) cast DMA f32->bf16, bit-matching the
                # numerics of the passing baseline final layer
                t5 = persist.tile([max(hi - lo, 16), 1024], bf16,
                                  tag=f"W5T_{lo}", name=f"W5T_{lo}")
                nc.gpsimd.dma_start(out=t5[:hi - lo, :], in_=W5T_d[i][:, :])
                W5T.append(t5)

            x0pool = tc.tile_pool(name="x0pool", bufs=1)
            x0T = x0pool.__enter__().tile([16, N], f32, tag="x0T")
            nc.sync.dma_start(out=x0T[:3, :], in_=xT_d[:, :])

            # ---- edge conv layers
            def edge_conv(l, c, o, xT, out_tag):
                notile = _ceil(o, 128)
                aug = c + 1 <= 65
                with ExitStack() as lx:
                    lwork = lx.enter_context(
                        tc.tile_pool(name=f"lwork{l}", bufs=2))
                    prep_ps = tc.tile_pool(name=f"prep_ps{l}", bufs=2,
                                           space="PSUM")
                    with prep_ps as pp:
                        sq = lwork.tile([max(c, 16), N], f32, tag="sq", bufs=1)
                        nc.scalar.square(sq[:c, :], xT[:c, :])
                        if aug:
                            lhs_sc = lwork.tile([c + 1, N], f32, tag="lhs_sc",
                                                bufs=1)
                            rhs_sc = lwork.tile([c + 1, N], f32, tag="rhs_sc",
                                                bufs=1)
                            aligned = (c % 32) == 0
                            if aligned:
                                nc.vector.tensor_scalar_mul(lhs_sc[:c, :],
                                                            xT[:c, :], 2.0)
                                nc.vector.memset(lhs_sc[c:c + 1, :], 1.0)
                                nc.scalar.copy(rhs_sc[:c, :], xT[:c, :])
                                negsq_dst = rhs_sc[c:c + 1, :]
                            else:
                                # engine APs must start 32-aligned: fill the
                                # ones row via full-height memset; negsq goes
                                # through a base-0 tile + contiguous SBUF DMA
                                nc.vector.memset(lhs_sc[:c + 1, :], 1.0)
                                nc.vector.tensor_scalar_mul(lhs_sc[:c, :],
                                                            xT[:c, :], 2.0)
                                nc.scalar.copy(rhs_sc[:c, :], xT[:c, :])
                                negsq = lwork.tile([1, N], f32, tag="negsq",
                                                   bufs=1)
                                negsq_dst = negsq[:, :]
                        else:
                            lhs_sc = lwork.tile([c, N], f32, tag="lhs_sc",
                                                bufs=1)
                            rhs_sc = xT
                            nc.vector.tensor_scalar_mul(lhs_sc[:c, :],
                                                        xT[:c, :], 2.0)
                            negsq = lwork.tile([1, N], f32, tag="negsq", bufs=1)
                            negsq_dst = negsq[:, :]
                        for ch in range(4):
                            cs = slice(ch * 512, (ch + 1) * 512)
                            nps = pp.tile([1, 512], f32, tag="negsq_ps")
                            nc.tensor.matmul(nps, lhsT=onesC[:c, :],
                                             rhs=sq[:c, cs],
                                             start=True, stop=True)
                            nc.scalar.mul(negsq_dst[:, cs], nps, -1.0)
                        if aug and not aligned:
                            nc.sync.dma_start(out=rhs_sc[c:c + 1, :],
                                              in_=negsq[:, :])

                    # --- u, v tiles; matmul units are emitted inside the
                    # block loop (all u at block 0 BEFORE the first gather
                    # emission, v at blocks 1-2 BEFORE the first tail) so the
                    # layer's first score matmuls aren't queued behind them on
                    # the PE. Program-order rule: every u_sb/v_sb write is
                    # emitted before its first reader.
                    uv_ps = lx.enter_context(
                        tc.tile_pool(name=f"uv_ps{l}", bufs=2, space="PSUM"))
                    u_sb = [lwork.tile([128, N], f32, tag=f"u_sb{t}",
                                       name=f"u_sb{t}", bufs=1)
                            for t in range(notile)]
                    v_sb = [lwork.tile([128, N], f32, tag=f"v_sb{t}",
                                       name=f"v_sb{t}", bufs=1)
                            for t in range(notile)]
                    rem = o - (notile - 1) * 128
                    if rem < 128:
                        nc.vector.memset(u_sb[notile - 1][rem:, :], 0.0)

                    def emit_uv(which, chunks):
                        for t in range(notile):
                            op = min(128, o - t * 128)
                            osl = slice(t * 128, t * 128 + op)
                            for ch in chunks:
                                cs = slice(ch * 512, (ch + 1) * 512)
                                if which == 'u':
                                    ups = uv_ps.tile([128, 512], f32,
                                                     tag="u_ps")
                                    nc.tensor.matmul(ups[:op, :],
                                                     lhsT=WaT[l][:c, osl],
                                                     rhs=xT[:c, cs],
                                                     start=True, stop=True)
                                    nc.scalar.copy(u_sb[t][:op, cs],
                                                   ups[:op, :])
                                else:
                                    vps = uv_ps.tile([128, 512], f32,
                                                     tag="v_ps")
                                    nc.tensor.matmul(vps[:op, :],
                                                     lhsT=WdT[l][:c, osl],
                                                     rhs=xT[:c, cs],
                                                     start=True, stop=True)
                                    nc.scalar.copy(v_sb[t][:op, cs],
                                                   vps[:op, :])

                    # --- block loop, software-pipelined: the per-block tail
                    # (gather -> k-max -> +v -> bn -> lrelu) is emitted LAG
                    # blocks behind its top-k, so the DVE top-k stream never
                    # stalls on the Pool-engine gathers, and Pool gathers for
                    # block b run while the DVE does top-k of block b+LAG.
                    # All fp32 tail math is bit-exact on any engine; the add
                    # and leaky-relu ride GPSIMD to keep the DVE lean.
                    xout = [persist.tile([128, N], f32, tag=f"{out_tag}_{t}",
                                         name=f"{out_tag}_{t}")
                            for t in range(notile)]
                    wrap_t = [lwork.tile([128, 160], u16, tag=f"wrap{b}",
                                         name=f"wrap{b}", bufs=1)
                              for b in range(NBLK)]
                    # Tail lag (blocks) behind the top-k: must cover the
                    # idx-DMA-bounce + gather latency (~22us) so the reduce is
                    # always ready when the static schedule reaches it. L4
                    # (notile=2) is SBUF-tight: shallower buffering there.
                    LAG = 3 if notile == 1 else 2
                    gath_bufs = (LAG + 1) * notile if notile == 1 else 5
                    sc_bufs = 3 if notile == 1 else 2

                    def block_gather(b):
                        # Pool-engine gathers, emitted right behind block b's
                        # top-k so the Pool queue always has gathers ahead of
                        # the cross-engine tail round-trips.
                        tiles = []
                        for t in range(notile):
                            gath = lwork.tile([128, 2560], f32, tag="gath",
                                              bufs=gath_bufs)
                            # walrus caps indirect_copy at 1024 indices
                            for lo in range(0, 2560, 1024):
                                hi = min(lo + 1024, 2560)
                                nc.gpsimd.indirect_copy(
                                    gath[:, lo:hi], u_sb[t],
                                    wrap_t[b][:, lo // 16:hi // 16], True)
                            tiles.append(gath)
                        return tiles

                    def block_tail(b, gtiles):
                        bsl = slice(b * 128, (b + 1) * 128)
                        for t in range(notile):
                            op = min(128, o - t * 128)
                            m_sb = lwork.tile([128, 128], f32, tag="m_sb")
                            nc.vector.tensor_reduce(
                                m_sb,
                                gtiles[t].rearrange("p (n k) -> p n k", k=KNN),
                                axis=mybir.AxisListType.X, op=Alu.max)
                            y = lwork.tile([128, 128], f32, tag="yb")
                            nc.vector.tensor_add(y[:op, :], m_sb[:op, :],
                                                 v_sb[t][:op, bsl])
                            ybn = lwork.tile([128, 128], f32, tag="ybn")
                            nc.scalar.activation(ybn[:op, :], y[:op, :],
                                                 Act.Identity,
                                                 bias=bn_sb[l][1][t][:op],
                                                 scale=bn_sb[l][0][t][:op])
                            nc.vector.scalar_tensor_tensor(
                                xout[t][:op, bsl], ybn[:op, :], NEG_SLOPE,
                                ybn[:op, :], op0=Alu.mult, op1=Alu.max)

                    pending = {}
                    with tc.tile_pool(name=f"sc_ps{l}", bufs=4,
                                      space="PSUM") as sp:
                        for b in range(NBLK):
                            bsl = slice(b * 128, (b + 1) * 128)
                            # scores per 1-bank PSUM chunk, staged to SBUF by
                            # exact f32 copies: each bank frees early so the
                            # PE streams ahead at full clock instead of
                            # idling into a low p-state
                            sc = lwork.tile([128, N], f32, tag="sc",
                                            bufs=sc_bufs)
                            for ch in range(4):
                                cs = slice(ch * 512, (ch + 1) * 512)
                                scps = sp.tile([128, 512], f32, tag="scps")
                                if aug:
                                    nc.tensor.matmul(scps,
                                                     lhsT=lhs_sc[:c + 1, bsl],
                                                     rhs=rhs_sc[:c + 1, cs],
                                                     start=True, stop=True)
                                else:
                                    nc.tensor.matmul(scps,
                                                     lhsT=lhs_sc[:c, bsl],
                                                     rhs=rhs_sc[:c, cs],
                                                     start=True, stop=False)
                                    nc.tensor.matmul(scps, lhsT=ones1,
                                                     rhs=negsq[:, cs],
                                                     start=False, stop=True)
                                nc.scalar.copy(sc[:, cs], scps)
                            idxb = lwork.tile([128, 24], u16, tag="idxb")
                            vals = lwork.tile([128, 8], f32, tag="vals")
                            nc.vector.max(vals, sc)
                            nc.vector.max_index(idxb[:, 0:8], vals, sc)
                            nc.vector.match_replace(sc, vals, sc, NEG_BIG)
                            nc.vector.max(vals, sc)
                            nc.vector.max_index(idxb[:, 8:16], vals, sc)
                            nc.vector.match_replace(sc, vals, sc, NEG_BIG)
                            nc.vector.max(vals, sc)
                            nc.vector.max_index(idxb[:, 16:24], vals, sc)

                            # n-major contiguous store: list[n*20+k] = idxb[n,k]
                            dst1 = bass.AP(tensor=list_d, offset=b * 2560,
                                           ap=[[KNN, 128], [1, KNN]])
                            nc.sync.dma_start(out=dst1, in_=idxb[:, 0:KNN])
                            # wrap read: wrapped[16g+p, s] = list[s*16+p]
                            for g in range(8):
                                src2 = bass.AP(tensor=list_d, offset=b * 2560,
                                               ap=[[1, 16], [16, 160]])
                                nc.sync.dma_start(
                                    out=wrap_t[b][g * 16:(g + 1) * 16, :],
                                    in_=src2)
                            # u/v interleave: all u before the first gather
                            # emission (gathers read u_sb), v split over
                            # blocks 1-2 (first tail reads v at b=LAG>=2)
                            if b == 0:
                                emit_uv('u', range(4))
                            elif b == 1:
                                emit_uv('v', (0, 1))
                            elif b == 2:
                                emit_uv('v', (2, 3))
                            pending[b] = block_gather(b)
                            if b >= LAG:
                                block_tail(b - LAG, pending.pop(b - LAG))
                        for b in range(NBLK - LAG, NBLK):
                            block_tail(b, pending.pop(b))
                return xout

            x1 = edge_conv(0, 3, 64, x0T, "x1")
            x0pool.__exit__(None, None, None)
            x2 = edge_conv(1, 64, 64, x1[0], "x2")
            x3 = edge_conv(2, 64, 128, x2[0], "x3")
            x4 = edge_conv(3, 128, 256, x3[0], "x4")

            # ---- final 1x1 conv (W5, f32) + BN + lrelu + global max/avg pool
            xcat_parts = [(x1[0], 64), (x2[0], 64), (x3[0], 128),
                          (x4[0], 128), (x4[1], 128)]
            inv_n = 1.0 / float(N)
            with tc.tile_pool(name="f_ps", bufs=4, space="PSUM") as fp, \
                 tc.tile_pool(name="fwork", bufs=2) as fw, \
                 tc.tile_pool(name="fb16", bufs=1) as fb:
                # bf16 staging + Identity/stt tail: numerically identical to
                # the passing baseline final layer
                xcb = []
                for i, (xp, ck) in enumerate(xcat_parts):
                    xtile = fb.tile([max(ck, 16), N], bf16, tag=f"xcb{i}",
                                    name=f"xcb{i}")
                    nc.vector.tensor_copy(xtile[:ck, :], xp[:ck, :])
                    xcb.append(xtile)
                for ot in range(8):
                    osl = slice(ot * 128, (ot + 1) * 128)
                    sums = fw.tile([128, 4], f32, tag="sums")
                    gmax = fw.tile([128, 512], f32, tag="gmax512")
                    for chn in range(4):
                        cs = slice(chn * 512, (chn + 1) * 512)
                        fps = fp.tile([128, 512], f32, tag="fps")
                        for i, (xp, ck) in enumerate(xcat_parts):
                            nc.tensor.matmul(fps, lhsT=W5T[i][:ck, osl],
                                             rhs=xcb[i][:ck, cs],
                                             start=(i == 0), stop=(i == 4))
                        ybn = fw.tile([128, 512], f32, tag="fybn")
                        nc.scalar.activation(ybn, fps, Act.Identity,
                                             bias=bn5_b[ot], scale=bn5_s[ot])
                        feat = fw.tile([128, 512], f32, tag="feat")
                        nc.vector.scalar_tensor_tensor(
                            feat, ybn, NEG_SLOPE, ybn,
                            op0=Alu.mult, op1=Alu.max,
                            accum_out=sums[:, chn:chn + 1])
                        if chn == 0:
                            nc.vector.tensor_copy(gmax, feat)
                        else:
                            nc.vector.tensor_max(gmax, gmax, feat)
                    gm = fw.tile([128, 1], f32, tag="gm")
                    nc.vector.tensor_reduce(gm, gmax, axis=mybir.AxisListType.X,
                                            op=Alu.max)
                    ga = fw.tile([128, 1], f32, tag="ga")
                    nc.vector.tensor_reduce(ga, sums, axis=mybir.AxisListType.X,
                                            op=Alu.add)
                    nc.vector.tensor_scalar_mul(ga, ga, inv_n)
                    nc.sync.dma_start(
                        out=bass.AP(tensor=out_d, offset=ot * 128,
                                    ap=[[1, 128]]),
                        in_=gm[:, :])
                    nc.sync.dma_start(
                        out=bass.AP(tensor=out_d, offset=1024 + ot * 128,
                                    ap=[[1, 128]]),
                        in_=ga[:, :])


def host_prepare(inputs):
    """Full inputs -> per-core input maps (host-side layout transforms)."""
    x = np.asarray(inputs["x"], dtype=np.float32)
    B = x.shape[0]
    shared = {}
    for l, (c, o) in enumerate(LAYERS):
        W = np.asarray(inputs[f"W{l+1}"], dtype=np.float32)
        Wa = W[:, :c]
        Wd = W[:, c:] - Wa
        shared[f"WaT{l+1}"] = np.ascontiguousarray(Wa.T)
        shared[f"WdT{l+1}"] = np.ascontiguousarray(Wd.T)
        bn = np.asarray(inputs[f"bn{l+1}"], dtype=np.float32)
        shared[f"bnT{l+1}"] = np.ascontiguousarray(bn.T)
    W5 = np.asarray(inputs["W5"], dtype=np.float32)
    for lo, hi in W5_CHUNKS:
        shared[f"W5T_{lo}"] = np.ascontiguousarray(W5[:, lo:hi].T)
    shared["bn5T"] = np.ascontiguousarray(
        np.asarray(inputs["bn5"], dtype=np.float32).T)
    return [dict(shared, xT=np.ascontiguousarray(x[b].T)) for b in range(B)]


_CACHED = {}


def _get_compiled():
    if "nc" not in _CACHED:
        nc = bacc.Bacc("TRN2", target_bir_lowering=False, debug=False,
                       num_devices=8)
        build_dgcnn(nc)
        nc.compile()
        _CACHED["nc"] = nc
    return _CACHED["nc"]


def kernel(**inputs):
    from concourse.bass_utils import run_bass_kernel_spmd
    nc = _get_compiled()
    in_maps = host_prepare(inputs)
    res = run_bass_kernel_spmd(nc, in_maps, list(range(len(in_maps))))
    out = np.stack([np.asarray(res.results[b]["out"]).reshape(-1)
                    for b in range(len(in_maps))], axis=0)
    return out.astype(np.float32)


# revision 55
# speedup vs baseline: 1.0345x; 1.0345x over previous
"""DGCNN encoder Bass kernel for Trainium2, data-parallel over batch on 8 cores.

Per core (one sample, x: (2048, 3)):
  4 EdgeConv layers + final 1x1 conv + global max/avg pool -> (2048,) output row.

Key algebraic restructuring (exact, since the BN scale gamma*rsqrt(var+eps) > 0
and leaky-relu is monotone):
  edgeconv(x)[n] = bnlrelu( max_{j in knn(n)} (Wa @ x_j)  +  (Wb - Wa) @ x_n )
with W = [Wa | Wb] acting on [x_j - x_n ; x_n].  This removes the k=20 factor
from all matmuls; only the top-20 selection and a row-gather + max remain.

kNN selection per 128-row block: score[n, j] = 2 x_n.x_j - |x_j|^2 (row-shifted
negated squared distance, same per-row order) via PE matmuls, then three
max8 / max_index / match_replace rounds on DVE for the exact top-20 set,
operating directly on the PSUM score tile (no SBUF copy).

Neighbor gather: indices are bounced through DRAM into the 16-partition-wrapped
layout the GPSIMD indirect_copy expects (same index list for every partition
group), gathering u^T = (Wa @ x)^T rows; max over k=20 via a GPSIMD
tensor_reduce (keeps the DVE free for the top-k passes).

Engine placement: all simple DMAs ride the HWDGE path (nc.sync / SP engine)
so the GPSIMD Pool engine only runs the indirect gathers + reduces; BN+lrelu
is a single Act-engine Prelu activation (alpha=0.2).

Static layout transforms (x -> x^T, W -> Wa^T / (Wb-Wa)^T, W5^T chunks,
bn -> bn^T) happen host-side in numpy: element-granularity strided DRAM DMAs
abort the NRT on this stack, and contiguous feeds make them unnecessary.
"""
import sys
sys.path.insert(0, '/opt/trn_rl_repo')

import numpy as np
import concourse.bass as bass
import concourse.bacc as bacc
import concourse.tile as tile
from concourse import mybir

f32 = mybir.dt.float32
bf16 = mybir.dt.bfloat16
u16 = mybir.dt.uint16
Alu = mybir.AluOpType
Act = mybir.ActivationFunctionType

N = 2048
NBLK = N // 128
KNN = 20
NEG_SLOPE = 0.2
BN_EPS = 1e-5
NEG_BIG = -1e30

# (C_in, O) per edge-conv layer
LAYERS = [(3, 64), (64, 64), (64, 128), (128, 256)]
# W5^T host-side chunks aligned to the xcat source tiles
W5_CHUNKS = [(0, 64), (64, 128), (128, 256), (256, 384), (384, 512)]


def _ceil(a, b):
    return (a + b - 1) // b


def build_dgcnn(nc):
    """Emit the full per-core DGCNN program into nc."""
    xT_d = nc.dram_tensor("xT", [3, N], f32, kind="ExternalInput")
    WaT_d = [nc.dram_tensor(f"WaT{l+1}", [c, o], f32, kind="ExternalInput")
             for l, (c, o) in enumerate(LAYERS)]
    WdT_d = [nc.dram_tensor(f"WdT{l+1}", [c, o], f32, kind="ExternalInput")
             for l, (c, o) in enumerate(LAYERS)]
    W5T_d = [nc.dram_tensor(f"W5T_{lo}", [hi - lo, 1024], f32,
                            kind="ExternalInput") for lo, hi in W5_CHUNKS]
    bnT_d = [nc.dram_tensor(f"bnT{l+1}", [o, 4], f32, kind="ExternalInput")
             for l, (c, o) in enumerate(LAYERS)]
    bn5T_d = nc.dram_tensor("bn5T", [1024, 4], f32, kind="ExternalInput")
    out_d = nc.dram_tensor("out", [2048], f32, kind="ExternalOutput")
    # DRAM bounce for the index wrap-relayout, n-major: list[n*20+k] = idx[n,k]
    list_d = nc.dram_tensor("idxlist", [NBLK * 128 * KNN], u16, kind="Internal")

    with tile.TileContext(nc) as tc:
        from contextlib import ExitStack
        ctx = ExitStack()
        with ctx:
            persist = ctx.enter_context(tc.tile_pool(name="persist", bufs=1))
            work = ctx.enter_context(tc.tile_pool(name="work", bufs=2))

            onesC = persist.tile([128, 1], f32, tag="onesC")
            nc.vector.memset(onesC, 1.0)
            ones1 = persist.tile([1, 128], f32, tag="ones1")
            nc.vector.memset(ones1, 1.0)
            eps_t = persist.tile([128, 1], f32, tag="eps")
            nc.vector.memset(eps_t, BN_EPS)

            # ---- bn param prep: (C, 4) rows [gamma, beta, mean, var] ->
            #      scale (C,1), bias (C,1) tiles per 128-channel chunk
            def prep_bn(bn_dram, channels, name):
                scales, biases = [], []
                for t in range(_ceil(channels, 128)):
                    p = min(128, channels - t * 128)
                    raw = work.tile([128, 4], f32, tag="bnraw")
                    src = bass.AP(tensor=bn_dram, offset=t * 128 * 4,
                                  ap=[[4, p], [1, 4]])
                    nc.sync.dma_start(out=raw[:p, :], in_=src)
                    s_t = persist.tile([128, 1], f32, tag=f"{name}_s{t}")
                    b_t = persist.tile([128, 1], f32, tag=f"{name}_b{t}")
                    tmp = work.tile([128, 1], f32, tag="bntmp")
                    nc.scalar.activation(tmp[:p], raw[:p, 3:4], Act.Sqrt,
                                         bias=eps_t[:p], scale=1.0)
                    nc.vector.reciprocal(tmp[:p], tmp[:p])
                    nc.vector.tensor_mul(s_t[:p], raw[:p, 0:1], tmp[:p])
                    nc.vector.tensor_mul(tmp[:p], raw[:p, 2:3], s_t[:p])
                    nc.vector.tensor_sub(b_t[:p], raw[:p, 1:2], tmp[:p])
                    scales.append(s_t)
                    biases.append(b_t)
                return scales, biases

            bn_sb = [prep_bn(bnT_d[l], o, f"bn{l}")
                     for l, (c, o) in enumerate(LAYERS)]
            bn5_s, bn5_b = prep_bn(bn5T_d, 1024, "bn5")

            # ---- weights (already transposed host-side)
            WaT, WdT = [], []
            for l, (c, o) in enumerate(LAYERS):
                wa = persist.tile([max(c, 16), o], f32, tag=f"WaT{l}",
                                  name=f"WaT{l}")
                nc.sync.dma_start(out=wa[:c, :], in_=WaT_d[l][:, :])
                wd = persist.tile([max(c, 16), o], f32, tag=f"WdT{l}",
                                  name=f"WdT{l}")
                nc.sync.dma_start(out=wd[:c, :], in_=WdT_d[l][:, :])
                WaT.append(wa)
                WdT.append(wd)
            W5T = []
            for i, (lo, hi) in enumerate(W5_CHUNKS):
                # gpsimd (SWDGE) cast DMA f32->bf16, bit-matching the
                # numerics of the passing baseline final layer
                t5 = persist.tile([max(hi - lo, 16), 1024], bf16,
                                  tag=f"W5T_{lo}", name=f"W5T_{lo}")
                nc.gpsimd.dma_start(out=t5[:hi - lo, :], in_=W5T_d[i][:, :])
                W5T.append(t5)

            x0pool = tc.tile_pool(name="x0pool", bufs=1)
            x0T = x0pool.__enter__().tile([16, N], f32, tag="x0T")
            nc.sync.dma_start(out=x0T[:3, :], in_=xT_d[:, :])

            # ---- edge conv layers
            def edge_conv(l, c, o, xT, out_tag):
                notile = _ceil(o, 128)
                aug = c + 1 <= 65
                with ExitStack() as lx:
                    lwork = lx.enter_context(
                        tc.tile_pool(name=f"lwork{l}", bufs=2))
                    prep_ps = tc.tile_pool(name=f"prep_ps{l}", bufs=2,
                                           space="PSUM")
                    with prep_ps as pp:
                        sq = lwork.tile([max(c, 16), N], f32, tag="sq", bufs=1)
                        nc.scalar.square(sq[:c, :], xT[:c, :])
                        if aug:
                            lhs_sc = lwork.tile([c + 1, N], f32, tag="lhs_sc",
                                                bufs=1)
                            rhs_sc = lwork.tile([c + 1, N], f32, tag="rhs_sc",
                                                bufs=1)
                            aligned = (c % 32) == 0
                            if aligned:
                                nc.vector.tensor_scalar_mul(lhs_sc[:c, :],
                                                            xT[:c, :], 2.0)
                                nc.vector.memset(lhs_sc[c:c + 1, :], 1.0)
                                nc.scalar.copy(rhs_sc[:c, :], xT[:c, :])
                                negsq_dst = rhs_sc[c:c + 1, :]
                            else:
                                # engine APs must start 32-aligned: fill the
                                # ones row via full-height memset; negsq goes
                                # through a base-0 tile + contiguous SBUF DMA
                                nc.vector.memset(lhs_sc[:c + 1, :], 1.0)
                                nc.vector.tensor_scalar_mul(lhs_sc[:c, :],
                                                            xT[:c, :], 2.0)
                                nc.scalar.copy(rhs_sc[:c, :], xT[:c, :])
                                negsq = lwork.tile([1, N], f32, tag="negsq",
                                                   bufs=1)
                                negsq_dst = negsq[:, :]
                        else:
                            lhs_sc = lwork.tile([c, N], f32, tag="lhs_sc",
                                                bufs=1)
                            rhs_sc = xT
                            nc.vector.tensor_scalar_mul(lhs_sc[:c, :],
                                                        xT[:c, :], 2.0)
                            negsq = lwork.tile([1, N], f32, tag="negsq", bufs=1)
                            negsq_dst = negsq[:, :]
                        for ch in range(4):
                            cs = slice(ch * 512, (ch + 1) * 512)
                            nps = pp.tile([1, 512], f32, tag="negsq_ps")
                            nc.tensor.matmul(nps, lhsT=onesC[:c, :],
                                             rhs=sq[:c, cs],
                                             start=True, stop=True)
                            nc.scalar.mul(negsq_dst[:, cs], nps, -1.0)
                        if aug and not aligned:
                            nc.sync.dma_start(out=rhs_sc[c:c + 1, :],
                                              in_=negsq[:, :])

                    # --- u, v tiles; matmul units are emitted inside the
                    # block loop (all u at block 0 BEFORE the first gather
                    # emission, v at blocks 1-2 BEFORE the first tail) so the
                    # layer's first score matmuls aren't queued behind them on
                    # the PE. Program-order rule: every u_sb/v_sb write is
                    # emitted before its first reader.
                    uv_ps = lx.enter_context(
                        tc.tile_pool(name=f"uv_ps{l}", bufs=2, space="PSUM"))
                    u_sb = [lwork.tile([128, N], f32, tag=f"u_sb{t}",
                                       name=f"u_sb{t}", bufs=1)
                            for t in range(notile)]
                    v_sb = [lwork.tile([128, N], f32, tag=f"v_sb{t}",
                                       name=f"v_sb{t}", bufs=1)
                            for t in range(notile)]
                    rem = o - (notile - 1) * 128
                    if rem < 128:
                        nc.vector.memset(u_sb[notile - 1][rem:, :], 0.0)

                    def emit_uv(which, chunks):
                        for t in range(notile):
                            op = min(128, o - t * 128)
                            osl = slice(t * 128, t * 128 + op)
                            for ch in chunks:
                                cs = slice(ch * 512, (ch + 1) * 512)
                                if which == 'u':
                                    ups = uv_ps.tile([128, 512], f32,
                                                     tag="u_ps")
                                    nc.tensor.matmul(ups[:op, :],
                                                     lhsT=WaT[l][:c, osl],
                                                     rhs=xT[:c, cs],
                                                     start=True, stop=True)
                                    nc.scalar.copy(u_sb[t][:op, cs],
                                                   ups[:op, :])
                                else:
                                    vps = uv_ps.tile([128, 512], f32,
                                                     tag="v_ps")
                                    nc.tensor.matmul(vps[:op, :],
                                                     lhsT=WdT[l][:c, osl],
                                                     rhs=xT[:c, cs],
                                                     start=True, stop=True)
                                    nc.scalar.copy(v_sb[t][:op, cs],
                                                   vps[:op, :])

                    # --- block loop, software-pipelined: the per-block tail
                    # (gather -> k-max -> +v -> bn -> lrelu) is emitted LAG
                    # blocks behind its top-k, so the DVE top-k stream never
                    # stalls on the Pool-engine gathers, and Pool gathers for
                    # block b run while the DVE does top-k of block b+LAG.
                    # All fp32 tail math is bit-exact on any engine; the add
                    # and leaky-relu ride GPSIMD to keep the DVE lean.
                    xout = [persist.tile([128, N], f32, tag=f"{out_tag}_{t}",
                                         name=f"{out_tag}_{t}")
                            for t in range(notile)]
                    wrap_t = [lwork.tile([128, 160], u16, tag=f"wrap{b}",
                                         name=f"wrap{b}", bufs=1)
                              for b in range(NBLK)]
                    # Tail lag (blocks) behind the top-k: must cover the
                    # idx-DMA-bounce + gather latency (~22us) so the reduce is
                    # always ready when the static schedule reaches it. L4
                    # (notile=2) is SBUF-tight: shallower buffering there.
                    LAG = 8 if notile == 1 else 4
                    gath_bufs = 6
                    sc_bufs = 3

                    def block_gather(b):
                        # Pool-engine gathers, emitted right behind block b's
                        # top-k so the Pool queue always has gathers ahead of
                        # the cross-engine tail round-trips.
                        tiles = []
                        for t in range(notile):
                            gath = lwork.tile([128, 2560], f32, tag="gath",
                                              bufs=gath_bufs)
                            # walrus caps indirect_copy at 1024 indices
                            for lo in range(0, 2560, 1024):
                                hi = min(lo + 1024, 2560)
                                nc.gpsimd.indirect_copy(
                                    gath[:, lo:hi], u_sb[t],
                                    wrap_t[b][:, lo // 16:hi // 16], True)
                            tiles.append(gath)
                        return tiles

                    def block_tail(b, gtiles):
                        bsl = slice(b * 128, (b + 1) * 128)
                        for t in range(notile):
                            op = min(128, o - t * 128)
                            m_sb = lwork.tile([128, 128], f32, tag="m_sb")
                            nc.vector.tensor_reduce(
                                m_sb,
                                gtiles[t].rearrange("p (n k) -> p n k", k=KNN),
                                axis=mybir.AxisListType.X, op=Alu.max)
                            y = lwork.tile([128, 128], f32, tag="yb")
                            nc.vector.tensor_add(y[:op, :], m_sb[:op, :],
                                                 v_sb[t][:op, bsl])
                            ybn = lwork.tile([128, 128], f32, tag="ybn")
                            nc.scalar.activation(ybn[:op, :], y[:op, :],
                                                 Act.Identity,
                                                 bias=bn_sb[l][1][t][:op],
                                                 scale=bn_sb[l][0][t][:op])
                            nc.vector.scalar_tensor_tensor(
                                xout[t][:op, bsl], ybn[:op, :], NEG_SLOPE,
                                ybn[:op, :], op0=Alu.mult, op1=Alu.max)

                    pending = {}
                    with tc.tile_pool(name=f"sc_ps{l}", bufs=4,
                                      space="PSUM") as sp:
                        for b in range(NBLK):
                            bsl = slice(b * 128, (b + 1) * 128)
                            # scores per 1-bank PSUM chunk, staged to SBUF by
                            # exact f32 copies: each bank frees early so the
                            # PE streams ahead at full clock instead of
                            # idling into a low p-state
                            sc = lwork.tile([128, N], f32, tag="sc",
                                            bufs=sc_bufs)
                            for ch in range(4):
                                cs = slice(ch * 512, (ch + 1) * 512)
                                scps = sp.tile([128, 512], f32, tag="scps")
                                if aug:
                                    nc.tensor.matmul(scps,
                                                     lhsT=lhs_sc[:c + 1, bsl],
                                                     rhs=rhs_sc[:c + 1, cs],
                                                     start=True, stop=True)
                                else:
                                    nc.tensor.matmul(scps,
                                                     lhsT=lhs_sc[:c, bsl],
                                                     rhs=rhs_sc[:c, cs],
                                                     start=True, stop=False)
                                    nc.tensor.matmul(scps, lhsT=ones1,
                                                     rhs=negsq[:, cs],
                                                     start=False, stop=True)
                                nc.scalar.copy(sc[:, cs], scps)
                            idxb = lwork.tile([128, 24], u16, tag="idxb")
                            vals = lwork.tile([128, 8], f32, tag="vals")
                            nc.vector.max(vals, sc)
                            nc.vector.max_index(idxb[:, 0:8], vals, sc)
                            nc.vector.match_replace(sc, vals, sc, NEG_BIG)
                            nc.vector.max(vals, sc)
                            nc.vector.max_index(idxb[:, 8:16], vals, sc)
                            nc.vector.match_replace(sc, vals, sc, NEG_BIG)
                            nc.vector.max(vals, sc)
                            nc.vector.max_index(idxb[:, 16:24], vals, sc)

                            # n-major contiguous store: list[n*20+k] = idxb[n,k]
                            dst1 = bass.AP(tensor=list_d, offset=b * 2560,
                                           ap=[[KNN, 128], [1, KNN]])
                            nc.sync.dma_start(out=dst1, in_=idxb[:, 0:KNN])
                            # wrap read: wrapped[16g+p, s] = list[s*16+p]
                            for g in range(8):
                                src2 = bass.AP(tensor=list_d, offset=b * 2560,
                                               ap=[[1, 16], [16, 160]])
                                nc.sync.dma_start(
                                    out=wrap_t[b][g * 16:(g + 1) * 16, :],
                                    in_=src2)
                            # u/v interleave: all u before the first gather
                            # emission (gathers read u_sb), v split over
                            # blocks 1-2 (first tail reads v at b=LAG>=2)
                            if b == 0:
                                emit_uv('u', range(4))
                            elif b == 1:
                                emit_uv('v', (0, 1))
                            elif b == 2:
                                emit_uv('v', (2, 3))
                            pending[b] = block_gather(b)
                            if b >= LAG:
                                block_tail(b - LAG, pending.pop(b - LAG))
                        for b in range(NBLK - LAG, NBLK):
                            block_tail(b, pending.pop(b))
                return xout

            x1 = edge_conv(0, 3, 64, x0T, "x1")
            x0pool.__exit__(None, None, None)
            x2 = edge_conv(1, 64, 64, x1[0], "x2")
            x3 = edge_conv(2, 64, 128, x2[0], "x3")
            x4 = edge_conv(3, 128, 256, x3[0], "x4")

            # ---- final 1x1 conv (W5, f32) + BN + lrelu + global max/avg pool
            xcat_parts = [(x1[0], 64), (x2[0], 64), (x3[0], 128),
                          (x4[0], 128), (x4[1], 128)]
            inv_n = 1.0 / float(N)
            with tc.tile_pool(name="f_ps", bufs=4, space="PSUM") as fp, \
                 tc.tile_pool(name="fwork", bufs=2) as fw, \
                 tc.tile_pool(name="fb16", bufs=1) as fb:
                # bf16 staging + Identity/stt tail: numerically identical to
                # the passing baseline final layer
                xcb = []
                for i, (xp, ck) in enumerate(xcat_parts):
                    xtile = fb.tile([max(ck, 16), N], bf16, tag=f"xcb{i}",
                                    name=f"xcb{i}")
                    nc.vector.tensor_copy(xtile[:ck, :], xp[:ck, :])
                    xcb.append(xtile)
                for ot in range(8):
                    osl = slice(ot * 128, (ot + 1) * 128)
                    sums = fw.tile([128, 4], f32, tag="sums")
                    gmax = fw.tile([128, 512], f32, tag="gmax512")
                    for chn in range(4):
                        cs = slice(chn * 512, (chn + 1) * 512)
                        fps = fp.tile([128, 512], f32, tag="fps")
                        for i, (xp, ck) in enumerate(xcat_parts):
                            nc.tensor.matmul(fps, lhsT=W5T[i][:ck, osl],
                                             rhs=xcb[i][:ck, cs],
                                             start=(i == 0), stop=(i == 4))
                        ybn = fw.tile([128, 512], f32, tag="fybn")
                        nc.scalar.activation(ybn, fps, Act.Identity,
                                             bias=bn5_b[ot], scale=bn5_s[ot])
                        feat = fw.tile([128, 512], f32, tag="feat")
                        nc.vector.scalar_tensor_tensor(
                            feat, ybn, NEG_SLOPE, ybn,
                            op0=Alu.mult, op1=Alu.max,
                            accum_out=sums[:, chn:chn + 1])
                        if chn == 0:
                            nc.vector.tensor_copy(gmax, feat)
                        else:
                            nc.vector.tensor_max(gmax, gmax, feat)
                    gm = fw.tile([128, 1], f32, tag="gm")
                    nc.vector.tensor_reduce(gm, gmax, axis=mybir.AxisListType.X,
                                            op=Alu.max)
                    ga = fw.tile([128, 1], f32, tag="ga")
                    nc.vector.tensor_reduce(ga, sums, axis=mybir.AxisListType.X,
                                            op=Alu.add)
                    nc.vector.tensor_scalar_mul(ga, ga, inv_n)
                    nc.sync.dma_start(
                        out=bass.AP(tensor=out_d, offset=ot * 128,
                                    ap=[[1, 128]]),
                        in_=gm[:, :])
                    nc.sync.dma_start(
                        out=bass.AP(tensor=out_d, offset=1024 + ot * 128,
                                    ap=[[1, 128]]),
                        in_=ga[:, :])


def host_prepare(inputs):
    """Full inputs -> per-core input maps (host-side layout transforms)."""
    x = np.asarray(inputs["x"], dtype=np.float32)
    B = x.shape[0]
    shared = {}
    for l, (c, o) in enumerate(LAYERS):
        W = np.asarray(inputs[f"W{l+1}"], dtype=np.float32)
        Wa = W[:, :c]
        Wd = W[:, c:] - Wa
        shared[f"WaT{l+1}"] = np.ascontiguousarray(Wa.T)
        shared[f"WdT{l+1}"] = np.ascontiguousarray(Wd.T)
        bn = np.asarray(inputs[f"bn{l+1}"], dtype=np.float32)
        shared[f"bnT{l+1}"] = np.ascontiguousarray(bn.T)
    W5 = np.asarray(inputs["W5"], dtype=np.float32)
    for lo, hi in W5_CHUNKS:
        shared[f"W5T_{lo}"] = np.ascontiguousarray(W5[:, lo:hi].T)
    shared["bn5T"] = np.ascontiguousarray(
        np.asarray(inputs["bn5"], dtype=np.float32).T)
    return [dict(shared, xT=np.ascontiguousarray(x[b].T)) for b in range(B)]


_CACHED = {}


def _get_compiled():
    if "nc" not in _CACHED:
        nc = bacc.Bacc("TRN2", target_bir_lowering=False, debug=False,
                       num_devices=8)
        build_dgcnn(nc)
        nc.compile()
        _CACHED["nc"] = nc
    return _CACHED["nc"]


def kernel(**inputs):
    from concourse.bass_utils import run_bass_kernel_spmd
    nc = _get_compiled()
    in_maps = host_prepare(inputs)
    res = run_bass_kernel_spmd(nc, in_maps, list(range(len(in_maps))))
    out = np.stack([np.asarray(res.results[b]["out"]).reshape(-1)
                    for b in range(len(in_maps))], axis=0)
    return out.astype(np.float32)


# revision 61
# speedup vs baseline: 1.0401x; 1.0054x over previous
"""DGCNN encoder Bass kernel for Trainium2, data-parallel over batch on 8 cores.

Per core (one sample, x: (2048, 3)):
  4 EdgeConv layers + final 1x1 conv + global max/avg pool -> (2048,) output row.

Key algebraic restructuring (exact, since the BN scale gamma*rsqrt(var+eps) > 0
and leaky-relu is monotone):
  edgeconv(x)[n] = bnlrelu( max_{j in knn(n)} (Wa @ x_j)  +  (Wb - Wa) @ x_n )
with W = [Wa | Wb] acting on [x_j - x_n ; x_n].  This removes the k=20 factor
from all matmuls; only the top-20 selection and a row-gather + max remain.

kNN selection per 128-row block: score[n, j] = 2 x_n.x_j - |x_j|^2 (row-shifted
negated squared distance, same per-row order) via PE matmuls, then three
max8 / max_index / match_replace rounds on DVE for the exact top-20 set,
operating directly on the PSUM score tile (no SBUF copy).

Neighbor gather: indices are bounced through DRAM into the 16-partition-wrapped
layout the GPSIMD indirect_copy expects (same index list for every partition
group), gathering u^T = (Wa @ x)^T rows; max over k=20 via a GPSIMD
tensor_reduce (keeps the DVE free for the top-k passes).

Engine placement: all simple DMAs ride the HWDGE path (nc.sync / SP engine)
so the GPSIMD Pool engine only runs the indirect gathers + reduces; BN+lrelu
is a single Act-engine Prelu activation (alpha=0.2).

Static layout transforms (x -> x^T, W -> Wa^T / (Wb-Wa)^T, W5^T chunks,
bn -> bn^T) happen host-side in numpy: element-granularity strided DRAM DMAs
abort the NRT on this stack, and contiguous feeds make them unnecessary.
"""
import sys
sys.path.insert(0, '/opt/trn_rl_repo')

import numpy as np
import concourse.bass as bass
import concourse.bacc as bacc
import concourse.tile as tile
from concourse import mybir

f32 = mybir.dt.float32
bf16 = mybir.dt.bfloat16
u16 = mybir.dt.uint16
Alu = mybir.AluOpType
Act = mybir.ActivationFunctionType

N = 2048
NBLK = N // 128
KNN = 20
NEG_SLOPE = 0.2
BN_EPS = 1e-5
NEG_BIG = -1e30

# (C_in, O) per edge-conv layer
LAYERS = [(3, 64), (64, 64), (64, 128), (128, 256)]
# W5^T host-side chunks aligned to the xcat source tiles
W5_CHUNKS = [(0, 64), (64, 128), (128, 256), (256, 384), (384, 512)]


def _ceil(a, b):
    return (a + b - 1) // b


def build_dgcnn(nc):
    """Emit the full per-core DGCNN program into nc."""
    xT_d = nc.dram_tensor("xT", [3, N], f32, kind="ExternalInput")
    WaT_d = [nc.dram_tensor(f"WaT{l+1}", [c, o], f32, kind="ExternalInput")
             for l, (c, o) in enumerate(LAYERS)]
    WdT_d = [nc.dram_tensor(f"WdT{l+1}", [c, o], f32, kind="ExternalInput")
             for l, (c, o) in enumerate(LAYERS)]
    W5T_d = [nc.dram_tensor(f"W5T_{lo}", [hi - lo, 1024], f32,
                            kind="ExternalInput") for lo, hi in W5_CHUNKS]
    bnT_d = [nc.dram_tensor(f"bnT{l+1}", [o, 4], f32, kind="ExternalInput")
             for l, (c, o) in enumerate(LAYERS)]
    bn5T_d = nc.dram_tensor("bn5T", [1024, 4], f32, kind="ExternalInput")
    out_d = nc.dram_tensor("out", [2048], f32, kind="ExternalOutput")
    # DRAM bounce for the index wrap-relayout, n-major: list[n*20+k] = idx[n,k]
    list_d = nc.dram_tensor("idxlist", [NBLK * 128 * KNN], u16, kind="Internal")

    with tile.TileContext(nc) as tc:
        from contextlib import ExitStack
        ctx = ExitStack()
        with ctx:
            persist = ctx.enter_context(tc.tile_pool(name="persist", bufs=1))
            work = ctx.enter_context(tc.tile_pool(name="work", bufs=2))

            onesC = persist.tile([128, 1], f32, tag="onesC")
            nc.vector.memset(onesC, 1.0)
            ones1 = persist.tile([1, 128], f32, tag="ones1")
            nc.vector.memset(ones1, 1.0)
            eps_t = persist.tile([128, 1], f32, tag="eps")
            nc.vector.memset(eps_t, BN_EPS)

            # ---- bn param prep: (C, 4) rows [gamma, beta, mean, var] ->
            #      scale (C,1), bias (C,1) tiles per 128-channel chunk
            def prep_bn(bn_dram, channels, name):
                scales, biases = [], []
                for t in range(_ceil(channels, 128)):
                    p = min(128, channels - t * 128)
                    raw = work.tile([128, 4], f32, tag="bnraw")
                    src = bass.AP(tensor=bn_dram, offset=t * 128 * 4,
                                  ap=[[4, p], [1, 4]])
                    nc.sync.dma_start(out=raw[:p, :], in_=src)
                    s_t = persist.tile([128, 1], f32, tag=f"{name}_s{t}")
                    b_t = persist.tile([128, 1], f32, tag=f"{name}_b{t}")
                    tmp = work.tile([128, 1], f32, tag="bntmp")
                    nc.scalar.activation(tmp[:p], raw[:p, 3:4], Act.Sqrt,
                                         bias=eps_t[:p], scale=1.0)
                    nc.vector.reciprocal(tmp[:p], tmp[:p])
                    nc.vector.tensor_mul(s_t[:p], raw[:p, 0:1], tmp[:p])
                    nc.vector.tensor_mul(tmp[:p], raw[:p, 2:3], s_t[:p])
                    nc.vector.tensor_sub(b_t[:p], raw[:p, 1:2], tmp[:p])
                    scales.append(s_t)
                    biases.append(b_t)
                return scales, biases

            bn_sb = [prep_bn(bnT_d[l], o, f"bn{l}")
                     for l, (c, o) in enumerate(LAYERS)]
            bn5_s, bn5_b = prep_bn(bn5T_d, 1024, "bn5")

            # ---- weights (already transposed host-side)
            WaT, WdT = [], []
            for l, (c, o) in enumerate(LAYERS):
                wa = persist.tile([max(c, 16), o], f32, tag=f"WaT{l}",
                                  name=f"WaT{l}")
                nc.sync.dma_start(out=wa[:c, :], in_=WaT_d[l][:, :])
                wd = persist.tile([max(c, 16), o], f32, tag=f"WdT{l}",
                                  name=f"WdT{l}")
                nc.sync.dma_start(out=wd[:c, :], in_=WdT_d[l][:, :])
                WaT.append(wa)
                WdT.append(wd)
            W5T = []
            for i, (lo, hi) in enumerate(W5_CHUNKS):
                # gpsimd (SWDGE) cast DMA f32->bf16, bit-matching the
                # numerics of the passing baseline final layer
                t5 = persist.tile([max(hi - lo, 16), 1024], bf16,
                                  tag=f"W5T_{lo}", name=f"W5T_{lo}")
                nc.gpsimd.dma_start(out=t5[:hi - lo, :], in_=W5T_d[i][:, :])
                W5T.append(t5)

            x0pool = tc.tile_pool(name="x0pool", bufs=1)
            x0T = x0pool.__enter__().tile([16, N], f32, tag="x0T")
            nc.sync.dma_start(out=x0T[:3, :], in_=xT_d[:, :])

            # ---- edge conv layers
            def edge_conv(l, c, o, xT, out_tag):
                notile = _ceil(o, 128)
                aug = c + 1 <= 65
                with ExitStack() as lx:
                    lwork = lx.enter_context(
                        tc.tile_pool(name=f"lwork{l}", bufs=2))
                    prep_ps = tc.tile_pool(name=f"prep_ps{l}", bufs=2,
                                           space="PSUM")
                    with prep_ps as pp:
                        sq = lwork.tile([max(c, 16), N], f32, tag="sq", bufs=1)
                        nc.scalar.square(sq[:c, :], xT[:c, :])
                        if aug:
                            lhs_sc = lwork.tile([c + 1, N], f32, tag="lhs_sc",
                                                bufs=1)
                            rhs_sc = lwork.tile([c + 1, N], f32, tag="rhs_sc",
                                                bufs=1)
                            aligned = (c % 32) == 0
                            if aligned:
                                nc.vector.tensor_scalar_mul(lhs_sc[:c, :],
                                                            xT[:c, :], 2.0)
                                nc.vector.memset(lhs_sc[c:c + 1, :], 1.0)
                                nc.scalar.copy(rhs_sc[:c, :], xT[:c, :])
                                negsq_dst = rhs_sc[c:c + 1, :]
                            else:
                                # engine APs must start 32-aligned: fill the
                                # ones row via full-height memset; negsq goes
                                # through a base-0 tile + contiguous SBUF DMA
                                nc.vector.memset(lhs_sc[:c + 1, :], 1.0)
                                nc.vector.tensor_scalar_mul(lhs_sc[:c, :],
                                                            xT[:c, :], 2.0)
                                nc.scalar.copy(rhs_sc[:c, :], xT[:c, :])
                                negsq = lwork.tile([1, N], f32, tag="negsq",
                                                   bufs=1)
                                negsq_dst = negsq[:, :]
                        else:
                            lhs_sc = lwork.tile([c, N], f32, tag="lhs_sc",
                                                bufs=1)
                            rhs_sc = xT
                            nc.vector.tensor_scalar_mul(lhs_sc[:c, :],
                                                        xT[:c, :], 2.0)
                            negsq = lwork.tile([1, N], f32, tag="negsq", bufs=1)
                            negsq_dst = negsq[:, :]
                        for ch in range(4):
                            cs = slice(ch * 512, (ch + 1) * 512)
                            nps = pp.tile([1, 512], f32, tag="negsq_ps")
                            nc.tensor.matmul(nps, lhsT=onesC[:c, :],
                                             rhs=sq[:c, cs],
                                             start=True, stop=True)
                            nc.scalar.mul(negsq_dst[:, cs], nps, -1.0)
                        if aug and not aligned:
                            nc.sync.dma_start(out=rhs_sc[c:c + 1, :],
                                              in_=negsq[:, :])

                    # --- u, v tiles; matmul units are emitted inside the
                    # block loop (all u at block 0 BEFORE the first gather
                    # emission, v at blocks 1-2 BEFORE the first tail) so the
                    # layer's first score matmuls aren't queued behind them on
                    # the PE. Program-order rule: every u_sb/v_sb write is
                    # emitted before its first reader.
                    uv_ps = lx.enter_context(
                        tc.tile_pool(name=f"uv_ps{l}", bufs=2, space="PSUM"))
                    u_sb = [lwork.tile([128, N], f32, tag=f"u_sb{t}",
                                       name=f"u_sb{t}", bufs=1)
                            for t in range(notile)]
                    v_sb = [lwork.tile([128, N], f32, tag=f"v_sb{t}",
                                       name=f"v_sb{t}", bufs=1)
                            for t in range(notile)]
                    rem = o - (notile - 1) * 128
                    if rem < 128:
                        nc.vector.memset(u_sb[notile - 1][rem:, :], 0.0)

                    def emit_uv(which, chunks):
                        for t in range(notile):
                            op = min(128, o - t * 128)
                            osl = slice(t * 128, t * 128 + op)
                            for ch in chunks:
                                cs = slice(ch * 512, (ch + 1) * 512)
                                if which == 'u':
                                    ups = uv_ps.tile([128, 512], f32,
                                                     tag="u_ps")
                                    nc.tensor.matmul(ups[:op, :],
                                                     lhsT=WaT[l][:c, osl],
                                                     rhs=xT[:c, cs],
                                                     start=True, stop=True)
                                    nc.scalar.copy(u_sb[t][:op, cs],
                                                   ups[:op, :])
                                else:
                                    vps = uv_ps.tile([128, 512], f32,
                                                     tag="v_ps")
                                    nc.tensor.matmul(vps[:op, :],
                                                     lhsT=WdT[l][:c, osl],
                                                     rhs=xT[:c, cs],
                                                     start=True, stop=True)
                                    nc.scalar.copy(v_sb[t][:op, cs],
                                                   vps[:op, :])

                    # --- block loop, software-pipelined: the per-block tail
                    # (gather -> k-max -> +v -> bn -> lrelu) is emitted LAG
                    # blocks behind its top-k, so the DVE top-k stream never
                    # stalls on the Pool-engine gathers, and Pool gathers for
                    # block b run while the DVE does top-k of block b+LAG.
                    # All fp32 tail math is bit-exact on any engine; the add
                    # and leaky-relu ride GPSIMD to keep the DVE lean.
                    xout = [persist.tile([128, N], f32, tag=f"{out_tag}_{t}",
                                         name=f"{out_tag}_{t}")
                            for t in range(notile)]
                    wrap_t = [lwork.tile([128, 160], u16, tag=f"wrap{b}",
                                         name=f"wrap{b}", bufs=1)
                              for b in range(NBLK)]
                    # Tail lag (blocks) behind the top-k: must cover the
                    # idx-DMA-bounce + gather latency (~22us) so the reduce is
                    # always ready when the static schedule reaches it. L4
                    # (notile=2) is SBUF-tight: shallower buffering there.
                    LAG = 10 if notile == 1 else 5
                    gath_bufs = 7 if notile == 1 else 6
                    sc_bufs = 2

                    def block_gather(b):
                        # Pool-engine gathers, emitted right behind block b's
                        # top-k so the Pool queue always has gathers ahead of
                        # the cross-engine tail round-trips.
                        tiles = []
                        for t in range(notile):
                            gath = lwork.tile([128, 2560], f32, tag="gath",
                                              bufs=gath_bufs)
                            # walrus caps indirect_copy at 1024 indices
                            for lo in range(0, 2560, 1024):
                                hi = min(lo + 1024, 2560)
                                nc.gpsimd.indirect_copy(
                                    gath[:, lo:hi], u_sb[t],
                                    wrap_t[b][:, lo // 16:hi // 16], True)
                            tiles.append(gath)
                        return tiles

                    def block_tail(b, gtiles):
                        bsl = slice(b * 128, (b + 1) * 128)
                        for t in range(notile):
                            op = min(128, o - t * 128)
                            m_sb = lwork.tile([128, 128], f32, tag="m_sb")
                            nc.vector.tensor_reduce(
                                m_sb,
                                gtiles[t].rearrange("p (n k) -> p n k", k=KNN),
                                axis=mybir.AxisListType.X, op=Alu.max)
                            y = lwork.tile([128, 128], f32, tag="yb")
                            nc.vector.tensor_add(y[:op, :], m_sb[:op, :],
                                                 v_sb[t][:op, bsl])
                            ybn = lwork.tile([128, 128], f32, tag="ybn")
                            nc.scalar.activation(ybn[:op, :], y[:op, :],
                                                 Act.Identity,
                                                 bias=bn_sb[l][1][t][:op],
                                                 scale=bn_sb[l][0][t][:op])
                            nc.vector.scalar_tensor_tensor(
                                xout[t][:op, bsl], ybn[:op, :], NEG_SLOPE,
                                ybn[:op, :], op0=Alu.mult, op1=Alu.max)

                    pending = {}
                    with tc.tile_pool(name=f"sc_ps{l}", bufs=4,
                                      space="PSUM") as sp:
                        for b in range(NBLK):
                            bsl = slice(b * 128, (b + 1) * 128)
                            # scores per 1-bank PSUM chunk, staged to SBUF by
                            # exact f32 copies: each bank frees early so the
                            # PE streams ahead at full clock instead of
                            # idling into a low p-state
                            sc = lwork.tile([128, N], f32, tag="sc",
                                            bufs=sc_bufs)
                            for ch in range(4):
                                cs = slice(ch * 512, (ch + 1) * 512)
                                scps = sp.tile([128, 512], f32, tag="scps")
                                if aug:
                                    nc.tensor.matmul(scps,
                                                     lhsT=lhs_sc[:c + 1, bsl],
                                                     rhs=rhs_sc[:c + 1, cs],
                                                     start=True, stop=True)
                                else:
                                    nc.tensor.matmul(scps,
                                                     lhsT=lhs_sc[:c, bsl],
                                                     rhs=rhs_sc[:c, cs],
                                                     start=True, stop=False)
                                    nc.tensor.matmul(scps, lhsT=ones1,
                                                     rhs=negsq[:, cs],
                                                     start=False, stop=True)
                                nc.scalar.copy(sc[:, cs], scps)
                            idxb = lwork.tile([128, 24], u16, tag="idxb")
                            vals = lwork.tile([128, 8], f32, tag="vals")
                            nc.vector.max(vals, sc)
                            nc.vector.max_index(idxb[:, 0:8], vals, sc)
                            nc.vector.match_replace(sc, vals, sc, NEG_BIG)
                            nc.vector.max(vals, sc)
                            nc.vector.max_index(idxb[:, 8:16], vals, sc)
                            nc.vector.match_replace(sc, vals, sc, NEG_BIG)
                            nc.vector.max(vals, sc)
                            nc.vector.max_index(idxb[:, 16:24], vals, sc)

                            # n-major contiguous store: list[n*20+k] = idxb[n,k]
                            dst1 = bass.AP(tensor=list_d, offset=b * 2560,
                                           ap=[[KNN, 128], [1, KNN]])
                            nc.sync.dma_start(out=dst1, in_=idxb[:, 0:KNN])
                            # wrap read: wrapped[16g+p, s] = list[s*16+p]
                            for g in range(8):
                                src2 = bass.AP(tensor=list_d, offset=b * 2560,
                                               ap=[[1, 16], [16, 160]])
                                nc.sync.dma_start(
                                    out=wrap_t[b][g * 16:(g + 1) * 16, :],
                                    in_=src2)
                            # u/v interleave: all u before the first gather
                            # emission (gathers read u_sb), v split over
                            # blocks 1-2 (first tail reads v at b=LAG>=2)
                            if b == 0:
                                emit_uv('u', range(4))
                            elif b == 1:
                                emit_uv('v', (0, 1))
                            elif b == 2:
                                emit_uv('v', (2, 3))
                            pending[b] = block_gather(b)
                            if b >= LAG:
                                block_tail(b - LAG, pending.pop(b - LAG))
                        for b in range(NBLK - LAG, NBLK):
                            block_tail(b, pending.pop(b))
                return xout

            x1 = edge_conv(0, 3, 64, x0T, "x1")
            x0pool.__exit__(None, None, None)
            x2 = edge_conv(1, 64, 64, x1[0], "x2")
            x3 = edge_conv(2, 64, 128, x2[0], "x3")
            x4 = edge_conv(3, 128, 256, x3[0], "x4")

            # ---- final 1x1 conv (W5, f32) + BN + lrelu + global max/avg pool
            xcat_parts = [(x1[0], 64), (x2[0], 64), (x3[0], 128),
                          (x4[0], 128), (x4[1], 128)]
            inv_n = 1.0 / float(N)
            with tc.tile_pool(name="f_ps", bufs=4, space="PSUM") as fp, \
                 tc.tile_pool(name="fwork", bufs=2) as fw, \
                 tc.tile_pool(name="fb16", bufs=1) as fb:
                # bf16 staging + Identity/stt tail: numerically identical to
                # the passing baseline final layer
                xcb = []
                for i, (xp, ck) in enumerate(xcat_parts):
                    xtile = fb.tile([max(ck, 16), N], bf16, tag=f"xcb{i}",
                                    name=f"xcb{i}")
                    nc.vector.tensor_copy(xtile[:ck, :], xp[:ck, :])
                    xcb.append(xtile)
                for ot in range(8):
                    osl = slice(ot * 128, (ot + 1) * 128)
                    sums = fw.tile([128, 4], f32, tag="sums")
                    gmax = fw.tile([128, 512], f32, tag="gmax512")
                    for chn in range(4):
                        cs = slice(chn * 512, (chn + 1) * 512)
                        fps = fp.tile([128, 512], f32, tag="fps")
                        for i, (xp, ck) in enumerate(xcat_parts):
                            nc.tensor.matmul(fps, lhsT=W5T[i][:ck, osl],
                                             rhs=xcb[i][:ck, cs],
                                             start=(i == 0), stop=(i == 4))
                        ybn = fw.tile([128, 512], f32, tag="fybn")
                        nc.scalar.activation(ybn, fps, Act.Identity,
                                             bias=bn5_b[ot], scale=bn5_s[ot])
                        feat = fw.tile([128, 512], f32, tag="feat")
                        nc.vector.scalar_tensor_tensor(
                            feat, ybn, NEG_SLOPE, ybn,
                            op0=Alu.mult, op1=Alu.max,
                            accum_out=sums[:, chn:chn + 1])
                        if chn == 0:
                            nc.vector.tensor_copy(gmax, feat)
                        else:
                            nc.vector.tensor_max(gmax, gmax, feat)
                    gm = fw.tile([128, 1], f32, tag="gm")
                    nc.vector.tensor_reduce(gm, gmax, axis=mybir.AxisListType.X,
                                            op=Alu.max)
                    ga = fw.tile([128, 1], f32, tag="ga")
                    nc.vector.tensor_reduce(ga, sums, axis=mybir.AxisListType.X,
                                            op=Alu.add)
                    nc.vector.tensor_scalar_mul(ga, ga, inv_n)
                    nc.sync.dma_start(
                        out=bass.AP(tensor=out_d, offset=ot * 128,
                                    ap=[[1, 128]]),
                        in_=gm[:, :])
                    nc.sync.dma_start(
                        out=bass.AP(tensor=out_d, offset=1024 + ot * 128,
                                    ap=[[1, 128]]),
                        in_=ga[:, :])


def host_prepare(inputs):
    """Full inputs -> per-core input maps (host-side layout transforms)."""
    x = np.asarray(inputs["x"], dtype=np.float32)
    B = x.shape[0]
    shared = {}
    for l, (c, o) in enumerate(LAYERS):
        W = np.asarray(inputs[f"W{l+1}"], dtype=np.float32)
        Wa = W[:, :c]
        Wd = W[:, c:] - Wa
        shared[f"WaT{l+1}"] = np.ascontiguousarray(Wa.T)
        shared[f"WdT{l+1}"] = np.ascontiguousarray(Wd.T)
        bn = np.asarray(inputs[f"bn{l+1}"], dtype=np.float32)
        shared[f"bnT{l+1}"] = np.ascontiguousarray(bn.T)
    W5 = np.asarray(inputs["W5"], dtype=np.float32)
    for lo, hi in W5_CHUNKS:
        shared[f"W5T_{lo}"] = np.ascontiguousarray(W5[:, lo:hi].T)
    shared["bn5T"] = np.ascontiguousarray(
        np.asarray(inputs["bn5"], dtype=np.float32).T)
    return [dict(shared, xT=np.ascontiguousarray(x[b].T)) for b in range(B)]


_CACHED = {}


def _get_compiled():
    if "nc" not in _CACHED:
        nc = bacc.Bacc("TRN2", target_bir_lowering=False, debug=False,
                       num_devices=8)
        build_dgcnn(nc)
        nc.compile()
        _CACHED["nc"] = nc
    return _CACHED["nc"]


def kernel(**inputs):
    from concourse.bass_utils import run_bass_kernel_spmd
    nc = _get_compiled()
    in_maps = host_prepare(inputs)
    res = run_bass_kernel_spmd(nc, in_maps, list(range(len(in_maps))))
    out = np.stack([np.asarray(res.results[b]["out"]).reshape(-1)
                    for b in range(len(in_maps))], axis=0)
    return out.astype(np.float32)


# revision 65
# speedup vs baseline: 1.0415x; 1.0013x over previous
"""DGCNN encoder Bass kernel for Trainium2, data-parallel over batch on 8 cores.

Per core (one sample, x: (2048, 3)):
  4 EdgeConv layers + final 1x1 conv + global max/avg pool -> (2048,) output row.

Key algebraic restructuring (exact, since the BN scale gamma*rsqrt(var+eps) > 0
and leaky-relu is monotone):
  edgeconv(x)[n] = bnlrelu( max_{j in knn(n)} (Wa @ x_j)  +  (Wb - Wa) @ x_n )
with W = [Wa | Wb] acting on [x_j - x_n ; x_n].  This removes the k=20 factor
from all matmuls; only the top-20 selection and a row-gather + max remain.

kNN selection per 128-row block: score[n, j] = 2 x_n.x_j - |x_j|^2 (row-shifted
negated squared distance, same per-row order) via PE matmuls, then three
max8 / max_index / match_replace rounds on DVE for the exact top-20 set,
operating directly on the PSUM score tile (no SBUF copy).

Neighbor gather: indices are bounced through DRAM into the 16-partition-wrapped
layout the GPSIMD indirect_copy expects (same index list for every partition
group), gathering u^T = (Wa @ x)^T rows; max over k=20 via a GPSIMD
tensor_reduce (keeps the DVE free for the top-k passes).

Engine placement: all simple DMAs ride the HWDGE path (nc.sync / SP engine)
so the GPSIMD Pool engine only runs the indirect gathers + reduces; BN+lrelu
is a single Act-engine Prelu activation (alpha=0.2).

Static layout transforms (x -> x^T, W -> Wa^T / (Wb-Wa)^T, W5^T chunks,
bn -> bn^T) happen host-side in numpy: element-granularity strided DRAM DMAs
abort the NRT on this stack, and contiguous feeds make them unnecessary.
"""
import sys
sys.path.insert(0, '/opt/trn_rl_repo')

import numpy as np
import concourse.bass as bass
import concourse.bacc as bacc
import concourse.tile as tile
from concourse import mybir

f32 = mybir.dt.float32
bf16 = mybir.dt.bfloat16
u16 = mybir.dt.uint16
Alu = mybir.AluOpType
Act = mybir.ActivationFunctionType

N = 2048
NBLK = N // 128
KNN = 20
NEG_SLOPE = 0.2
BN_EPS = 1e-5
NEG_BIG = -1e30

# (C_in, O) per edge-conv layer
LAYERS = [(3, 64), (64, 64), (64, 128), (128, 256)]
# W5^T host-side chunks aligned to the xcat source tiles
W5_CHUNKS = [(0, 64), (64, 128), (128, 256), (256, 384), (384, 512)]


def _ceil(a, b):
    return (a + b - 1) // b


def build_dgcnn(nc):
    """Emit the full per-core DGCNN program into nc."""
    xT_d = nc.dram_tensor("xT", [3, N], f32, kind="ExternalInput")
    WaT_d = [nc.dram_tensor(f"WaT{l+1}", [c, o], f32, kind="ExternalInput")
             for l, (c, o) in enumerate(LAYERS)]
    WdT_d = [nc.dram_tensor(f"WdT{l+1}", [c, o], f32, kind="ExternalInput")
             for l, (c, o) in enumerate(LAYERS)]
    W5T_d = [nc.dram_tensor(f"W5T_{lo}", [hi - lo, 1024], f32,
                            kind="ExternalInput") for lo, hi in W5_CHUNKS]
    bnT_d = [nc.dram_tensor(f"bnT{l+1}", [o, 4], f32, kind="ExternalInput")
             for l, (c, o) in enumerate(LAYERS)]
    bn5T_d = nc.dram_tensor("bn5T", [1024, 4], f32, kind="ExternalInput")
    out_d = nc.dram_tensor("out", [2048], f32, kind="ExternalOutput")
    # DRAM bounce for the index wrap-relayout, n-major: list[n*20+k] = idx[n,k]
    list_d = nc.dram_tensor("idxlist", [NBLK * 128 * KNN], u16, kind="Internal")

    with tile.TileContext(nc) as tc:
        from contextlib import ExitStack
        ctx = ExitStack()
        with ctx:
            persist = ctx.enter_context(tc.tile_pool(name="persist", bufs=1))
            work = ctx.enter_context(tc.tile_pool(name="work", bufs=2))

            onesC = persist.tile([128, 1], f32, tag="onesC")
            nc.vector.memset(onesC, 1.0)
            ones1 = persist.tile([1, 128], f32, tag="ones1")
            nc.vector.memset(ones1, 1.0)
            eps_t = persist.tile([128, 1], f32, tag="eps")
            nc.vector.memset(eps_t, BN_EPS)

            # ---- bn param prep: (C, 4) rows [gamma, beta, mean, var] ->
            #      scale (C,1), bias (C,1) tiles per 128-channel chunk
            def prep_bn(bn_dram, channels, name):
                scales, biases = [], []
                for t in range(_ceil(channels, 128)):
                    p = min(128, channels - t * 128)
                    raw = work.tile([128, 4], f32, tag="bnraw")
                    src = bass.AP(tensor=bn_dram, offset=t * 128 * 4,
                                  ap=[[4, p], [1, 4]])
                    nc.sync.dma_start(out=raw[:p, :], in_=src)
                    s_t = persist.tile([128, 1], f32, tag=f"{name}_s{t}")
                    b_t = persist.tile([128, 1], f32, tag=f"{name}_b{t}")
                    tmp = work.tile([128, 1], f32, tag="bntmp")
                    nc.scalar.activation(tmp[:p], raw[:p, 3:4], Act.Sqrt,
                                         bias=eps_t[:p], scale=1.0)
                    nc.vector.reciprocal(tmp[:p], tmp[:p])
                    nc.vector.tensor_mul(s_t[:p], raw[:p, 0:1], tmp[:p])
                    nc.vector.tensor_mul(tmp[:p], raw[:p, 2:3], s_t[:p])
                    nc.vector.tensor_sub(b_t[:p], raw[:p, 1:2], tmp[:p])
                    scales.append(s_t)
                    biases.append(b_t)
                return scales, biases

            bn_sb = [prep_bn(bnT_d[l], o, f"bn{l}")
                     for l, (c, o) in enumerate(LAYERS)]
            bn5_s, bn5_b = prep_bn(bn5T_d, 1024, "bn5")

            # ---- weights (already transposed host-side)
            WaT, WdT = [], []
            for l, (c, o) in enumerate(LAYERS):
                wa = persist.tile([max(c, 16), o], f32, tag=f"WaT{l}",
                                  name=f"WaT{l}")
                nc.sync.dma_start(out=wa[:c, :], in_=WaT_d[l][:, :])
                wd = persist.tile([max(c, 16), o], f32, tag=f"WdT{l}",
                                  name=f"WdT{l}")
                nc.sync.dma_start(out=wd[:c, :], in_=WdT_d[l][:, :])
                WaT.append(wa)
                WdT.append(wd)
            W5T = []
            for i, (lo, hi) in enumerate(W5_CHUNKS):
                # gpsimd (SWDGE) cast DMA f32->bf16, bit-matching the
                # numerics of the passing baseline final layer
                t5 = persist.tile([max(hi - lo, 16), 1024], bf16,
                                  tag=f"W5T_{lo}", name=f"W5T_{lo}")
                nc.gpsimd.dma_start(out=t5[:hi - lo, :], in_=W5T_d[i][:, :])
                W5T.append(t5)

            x0pool = tc.tile_pool(name="x0pool", bufs=1)
            x0T = x0pool.__enter__().tile([16, N], f32, tag="x0T")
            nc.sync.dma_start(out=x0T[:3, :], in_=xT_d[:, :])

            # ---- edge conv layers
            def edge_conv(l, c, o, xT, out_tag):
                notile = _ceil(o, 128)
                aug = c + 1 <= 65
                with ExitStack() as lx:
                    lwork = lx.enter_context(
                        tc.tile_pool(name=f"lwork{l}", bufs=2))
                    prep_ps = tc.tile_pool(name=f"prep_ps{l}", bufs=2,
                                           space="PSUM")
                    with prep_ps as pp:
                        sq = lwork.tile([max(c, 16), N], f32, tag="sq", bufs=1)
                        nc.scalar.square(sq[:c, :], xT[:c, :])
                        if aug:
                            lhs_sc = lwork.tile([c + 1, N], f32, tag="lhs_sc",
                                                bufs=1)
                            rhs_sc = lwork.tile([c + 1, N], f32, tag="rhs_sc",
                                                bufs=1)
                            aligned = (c % 32) == 0
                            if aligned:
                                nc.vector.tensor_scalar_mul(lhs_sc[:c, :],
                                                            xT[:c, :], 2.0)
                                nc.vector.memset(lhs_sc[c:c + 1, :], 1.0)
                                nc.scalar.copy(rhs_sc[:c, :], xT[:c, :])
                                negsq_dst = rhs_sc[c:c + 1, :]
                            else:
                                # engine APs must start 32-aligned: fill the
                                # ones row via full-height memset; negsq goes
                                # through a base-0 tile + contiguous SBUF DMA
                                nc.vector.memset(lhs_sc[:c + 1, :], 1.0)
                                nc.vector.tensor_scalar_mul(lhs_sc[:c, :],
                                                            xT[:c, :], 2.0)
                                nc.scalar.copy(rhs_sc[:c, :], xT[:c, :])
                                negsq = lwork.tile([1, N], f32, tag="negsq",
                                                   bufs=1)
                                negsq_dst = negsq[:, :]
                        else:
                            lhs_sc = lwork.tile([c, N], f32, tag="lhs_sc",
                                                bufs=1)
                            rhs_sc = xT
                            nc.vector.tensor_scalar_mul(lhs_sc[:c, :],
                                                        xT[:c, :], 2.0)
                            negsq = lwork.tile([1, N], f32, tag="negsq", bufs=1)
                            negsq_dst = negsq[:, :]
                        for ch in range(4):
                            cs = slice(ch * 512, (ch + 1) * 512)
                            nps = pp.tile([1, 512], f32, tag="negsq_ps")
                            nc.tensor.matmul(nps, lhsT=onesC[:c, :],
                                             rhs=sq[:c, cs],
                                             start=True, stop=True)
                            nc.scalar.mul(negsq_dst[:, cs], nps, -1.0)
                        if aug and not aligned:
                            nc.sync.dma_start(out=rhs_sc[c:c + 1, :],
                                              in_=negsq[:, :])

                    # --- u, v tiles; matmul units are emitted inside the
                    # block loop (all u at block 0 BEFORE the first gather
                    # emission, v at blocks 1-2 BEFORE the first tail) so the
                    # layer's first score matmuls aren't queued behind them on
                    # the PE. Program-order rule: every u_sb/v_sb write is
                    # emitted before its first reader.
                    uv_ps = lx.enter_context(
                        tc.tile_pool(name=f"uv_ps{l}", bufs=2, space="PSUM"))
                    u_sb = [lwork.tile([128, N], f32, tag=f"u_sb{t}",
                                       name=f"u_sb{t}", bufs=1)
                            for t in range(notile)]
                    v_sb = [lwork.tile([128, N], f32, tag=f"v_sb{t}",
                                       name=f"v_sb{t}", bufs=1)
                            for t in range(notile)]
                    rem = o - (notile - 1) * 128
                    if rem < 128:
                        nc.vector.memset(u_sb[notile - 1][rem:, :], 0.0)

                    def emit_uv(which, chunks):
                        for t in range(notile):
                            op = min(128, o - t * 128)
                            osl = slice(t * 128, t * 128 + op)
                            for ch in chunks:
                                cs = slice(ch * 512, (ch + 1) * 512)
                                if which == 'u':
                                    ups = uv_ps.tile([128, 512], f32,
                                                     tag="u_ps")
                                    nc.tensor.matmul(ups[:op, :],
                                                     lhsT=WaT[l][:c, osl],
                                                     rhs=xT[:c, cs],
                                                     start=True, stop=True)
                                    nc.scalar.copy(u_sb[t][:op, cs],
                                                   ups[:op, :])
                                else:
                                    vps = uv_ps.tile([128, 512], f32,
                                                     tag="v_ps")
                                    nc.tensor.matmul(vps[:op, :],
                                                     lhsT=WdT[l][:c, osl],
                                                     rhs=xT[:c, cs],
                                                     start=True, stop=True)
                                    nc.scalar.copy(v_sb[t][:op, cs],
                                                   vps[:op, :])

                    # --- block loop, software-pipelined: the per-block tail
                    # (gather -> k-max -> +v -> bn -> lrelu) is emitted LAG
                    # blocks behind its top-k, so the DVE top-k stream never
                    # stalls on the Pool-engine gathers, and Pool gathers for
                    # block b run while the DVE does top-k of block b+LAG.
                    # All fp32 tail math is bit-exact on any engine; the add
                    # and leaky-relu ride GPSIMD to keep the DVE lean.
                    xout = [persist.tile([128, N], f32, tag=f"{out_tag}_{t}",
                                         name=f"{out_tag}_{t}")
                            for t in range(notile)]
                    wrap_t = [lwork.tile([128, 160], u16, tag=f"wrap{b}",
                                         name=f"wrap{b}", bufs=1)
                              for b in range(NBLK)]
                    # Tail lag (blocks) behind the top-k: must cover the
                    # idx-DMA-bounce + gather latency (~22us) so the reduce is
                    # always ready when the static schedule reaches it. L4
                    # (notile=2) is SBUF-tight: shallower buffering there.
                    LAG = 10 if notile == 1 else 5
                    gath_bufs = 7
                    sc_bufs = 2

                    def block_gather(b):
                        # Pool-engine gathers, emitted right behind block b's
                        # top-k so the Pool queue always has gathers ahead of
                        # the cross-engine tail round-trips.
                        tiles = []
                        for t in range(notile):
                            gath = lwork.tile([128, 2560], f32, tag="gath",
                                              bufs=gath_bufs)
                            # walrus caps indirect_copy at 1024 indices
                            for lo in range(0, 2560, 1024):
                                hi = min(lo + 1024, 2560)
                                nc.gpsimd.indirect_copy(
                                    gath[:, lo:hi], u_sb[t],
                                    wrap_t[b][:, lo // 16:hi // 16], True)
                            tiles.append(gath)
                        return tiles

                    def block_tail(b, gtiles):
                        bsl = slice(b * 128, (b + 1) * 128)
                        for t in range(notile):
                            op = min(128, o - t * 128)
                            m_sb = lwork.tile([128, 128], f32, tag="m_sb")
                            nc.vector.tensor_reduce(
                                m_sb,
                                gtiles[t].rearrange("p (n k) -> p n k", k=KNN),
                                axis=mybir.AxisListType.X, op=Alu.max)
                            y = lwork.tile([128, 128], f32, tag="yb")
                            nc.vector.tensor_add(y[:op, :], m_sb[:op, :],
                                                 v_sb[t][:op, bsl])
                            ybn = lwork.tile([128, 128], f32, tag="ybn")
                            nc.scalar.activation(ybn[:op, :], y[:op, :],
                                                 Act.Identity,
                                                 bias=bn_sb[l][1][t][:op],
                                                 scale=bn_sb[l][0][t][:op])
                            nc.vector.scalar_tensor_tensor(
                                xout[t][:op, bsl], ybn[:op, :], NEG_SLOPE,
                                ybn[:op, :], op0=Alu.mult, op1=Alu.max)

                    pending = {}
                    with tc.tile_pool(name=f"sc_ps{l}", bufs=4,
                                      space="PSUM") as sp:
                        for b in range(NBLK):
                            bsl = slice(b * 128, (b + 1) * 128)
                            # scores per 1-bank PSUM chunk, staged to SBUF by
                            # exact f32 copies: each bank frees early so the
                            # PE streams ahead at full clock instead of
                            # idling into a low p-state
                            sc = lwork.tile([128, N], f32, tag="sc",
                                            bufs=sc_bufs)
                            for ch in range(4):
                                cs = slice(ch * 512, (ch + 1) * 512)
                                scps = sp.tile([128, 512], f32, tag="scps")
                                if aug:
                                    nc.tensor.matmul(scps,
                                                     lhsT=lhs_sc[:c + 1, bsl],
                                                     rhs=rhs_sc[:c + 1, cs],
                                                     start=True, stop=True)
                                else:
                                    nc.tensor.matmul(scps,
                                                     lhsT=lhs_sc[:c, bsl],
                                                     rhs=rhs_sc[:c, cs],
                                                     start=True, stop=False)
                                    nc.tensor.matmul(scps, lhsT=ones1,
                                                     rhs=negsq[:, cs],
                                                     start=False, stop=True)
                                nc.scalar.copy(sc[:, cs], scps)
                            idxb = lwork.tile([128, 24], u16, tag="idxb")
                            vals = lwork.tile([128, 8], f32, tag="vals")
                            nc.vector.max(vals, sc)
                            nc.vector.max_index(idxb[:, 0:8], vals, sc)
                            nc.vector.match_replace(sc, vals, sc, NEG_BIG)
                            nc.vector.max(vals, sc)
                            nc.vector.max_index(idxb[:, 8:16], vals, sc)
                            nc.vector.match_replace(sc, vals, sc, NEG_BIG)
                            nc.vector.max(vals, sc)
                            nc.vector.max_index(idxb[:, 16:24], vals, sc)

                            # n-major contiguous store: list[n*20+k] = idxb[n,k]
                            dst1 = bass.AP(tensor=list_d, offset=b * 2560,
                                           ap=[[KNN, 128], [1, KNN]])
                            nc.sync.dma_start(out=dst1, in_=idxb[:, 0:KNN])
                            # wrap read: wrapped[16g+p, s] = list[s*16+p]
                            for g in range(8):
                                src2 = bass.AP(tensor=list_d, offset=b * 2560,
                                               ap=[[1, 16], [16, 160]])
                                nc.sync.dma_start(
                                    out=wrap_t[b][g * 16:(g + 1) * 16, :],
                                    in_=src2)
                            # u/v interleave: all u before the first gather
                            # emission (gathers read u_sb), v split over
                            # blocks 1-2 (first tail reads v at b=LAG>=2)
                            if b == 0:
                                emit_uv('u', range(4))
                            elif b == 1:
                                emit_uv('v', (0, 1))
                            elif b == 2:
                                emit_uv('v', (2, 3))
                            pending[b] = block_gather(b)
                            if b >= LAG:
                                block_tail(b - LAG, pending.pop(b - LAG))
                        for b in range(NBLK - LAG, NBLK):
                            block_tail(b, pending.pop(b))
                return xout

            x1 = edge_conv(0, 3, 64, x0T, "x1")
            x0pool.__exit__(None, None, None)
            x2 = edge_conv(1, 64, 64, x1[0], "x2")
            x3 = edge_conv(2, 64, 128, x2[0], "x3")
            x4 = edge_conv(3, 128, 256, x3[0], "x4")

            # ---- final 1x1 conv (W5, f32) + BN + lrelu + global max/avg pool
            xcat_parts = [(x1[0], 64), (x2[0], 64), (x3[0], 128),
                          (x4[0], 128), (x4[1], 128)]
            inv_n = 1.0 / float(N)
            with tc.tile_pool(name="f_ps", bufs=4, space="PSUM") as fp, \
                 tc.tile_pool(name="fwork", bufs=2) as fw, \
                 tc.tile_pool(name="fb16", bufs=1) as fb:
                # bf16 staging + Identity/stt tail: numerically identical to
                # the passing baseline final layer
                xcb = []
                for i, (xp, ck) in enumerate(xcat_parts):
                    xtile = fb.tile([max(ck, 16), N], bf16, tag=f"xcb{i}",
                                    name=f"xcb{i}")
                    nc.vector.tensor_copy(xtile[:ck, :], xp[:ck, :])
                    xcb.append(xtile)
                for ot in range(8):
                    osl = slice(ot * 128, (ot + 1) * 128)
                    sums = fw.tile([128, 4], f32, tag="sums")
                    gmax = fw.tile([128, 512], f32, tag="gmax512")
                    for chn in range(4):
                        cs = slice(chn * 512, (chn + 1) * 512)
                        fps = fp.tile([128, 512], f32, tag="fps")
                        for i, (xp, ck) in enumerate(xcat_parts):
                            nc.tensor.matmul(fps, lhsT=W5T[i][:ck, osl],
                                             rhs=xcb[i][:ck, cs],
                                             start=(i == 0), stop=(i == 4))
                        ybn = fw.tile([128, 512], f32, tag="fybn")
                        nc.scalar.activation(ybn, fps, Act.Identity,
                                             bias=bn5_b[ot], scale=bn5_s[ot])
                        feat = fw.tile([128, 512], f32, tag="feat")
                        nc.vector.scalar_tensor_tensor(
                            feat, ybn, NEG_SLOPE, ybn,
                            op0=Alu.mult, op1=Alu.max,
                            accum_out=sums[:, chn:chn + 1])
                        if chn == 0:
                            nc.vector.tensor_copy(gmax, feat)
                        else:
                            nc.vector.tensor_max(gmax, gmax, feat)
                    gm = fw.tile([128, 1], f32, tag="gm")
                    nc.vector.tensor_reduce(gm, gmax, axis=mybir.AxisListType.X,
                                            op=Alu.max)
                    ga = fw.tile([128, 1], f32, tag="ga")
                    nc.vector.tensor_reduce(ga, sums, axis=mybir.AxisListType.X,
                                            op=Alu.add)
                    nc.vector.tensor_scalar_mul(ga, ga, inv_n)
                    nc.sync.dma_start(
                        out=bass.AP(tensor=out_d, offset=ot * 128,
                                    ap=[[1, 128]]),
                        in_=gm[:, :])
                    nc.sync.dma_start(
                        out=bass.AP(tensor=out_d, offset=1024 + ot * 128,
                                    ap=[[1, 128]]),
                        in_=ga[:, :])


def host_prepare(inputs):
    """Full inputs -> per-core input maps (host-side layout transforms)."""
    x = np.asarray(inputs["x"], dtype=np.float32)
    B = x.shape[0]
    shared = {}
    for l, (c, o) in enumerate(LAYERS):
        W = np.asarray(inputs[f"W{l+1}"], dtype=np.float32)
        Wa = W[:, :c]
        Wd = W[:, c:] - Wa
        shared[f"WaT{l+1}"] = np.ascontiguousarray(Wa.T)
        shared[f"WdT{l+1}"] = np.ascontiguousarray(Wd.T)
        bn = np.asarray(inputs[f"bn{l+1}"], dtype=np.float32)
        shared[f"bnT{l+1}"] = np.ascontiguousarray(bn.T)
    W5 = np.asarray(inputs["W5"], dtype=np.float32)
    for lo, hi in W5_CHUNKS:
        shared[f"W5T_{lo}"] = np.ascontiguousarray(W5[:, lo:hi].T)
    shared["bn5T"] = np.ascontiguousarray(
        np.asarray(inputs["bn5"], dtype=np.float32).T)
    return [dict(shared, xT=np.ascontiguousarray(x[b].T)) for b in range(B)]


_CACHED = {}


def _get_compiled():
    if "nc" not in _CACHED:
        nc = bacc.Bacc("TRN2", target_bir_lowering=False, debug=False,
                       num_devices=8)
        build_dgcnn(nc)
        nc.compile()
        _CACHED["nc"] = nc
    return _CACHED["nc"]


def kernel(**inputs):
    from concourse.bass_utils import run_bass_kernel_spmd
    nc = _get_compiled()
    in_maps = host_prepare(inputs)
    res = run_bass_kernel_spmd(nc, in_maps, list(range(len(in_maps))))
    out = np.stack([np.asarray(res.results[b]["out"]).reshape(-1)
                    for b in range(len(in_maps))], axis=0)
    return out.astype(np.float32)


# revision 66
# speedup vs baseline: 1.0456x; 1.0039x over previous
"""DGCNN encoder Bass kernel for Trainium2, data-parallel over batch on 8 cores.

Per core (one sample, x: (2048, 3)):
  4 EdgeConv layers + final 1x1 conv + global max/avg pool -> (2048,) output row.

Key algebraic restructuring (exact, since the BN scale gamma*rsqrt(var+eps) > 0
and leaky-relu is monotone):
  edgeconv(x)[n] = bnlrelu( max_{j in knn(n)} (Wa @ x_j)  +  (Wb - Wa) @ x_n )
with W = [Wa | Wb] acting on [x_j - x_n ; x_n].  This removes the k=20 factor
from all matmuls; only the top-20 selection and a row-gather + max remain.

kNN selection per 128-row block: score[n, j] = 2 x_n.x_j - |x_j|^2 (row-shifted
negated squared distance, same per-row order) via PE matmuls, then three
max8 / max_index / match_replace rounds on DVE for the exact top-20 set,
operating directly on the PSUM score tile (no SBUF copy).

Neighbor gather: indices are bounced through DRAM into the 16-partition-wrapped
layout the GPSIMD indirect_copy expects (same index list for every partition
group), gathering u^T = (Wa @ x)^T rows; max over k=20 via a GPSIMD
tensor_reduce (keeps the DVE free for the top-k passes).

Engine placement: all simple DMAs ride the HWDGE path (nc.sync / SP engine)
so the GPSIMD Pool engine only runs the indirect gathers + reduces; BN+lrelu
is a single Act-engine Prelu activation (alpha=0.2).

Static layout transforms (x -> x^T, W -> Wa^T / (Wb-Wa)^T, W5^T chunks,
bn -> bn^T) happen host-side in numpy: element-granularity strided DRAM DMAs
abort the NRT on this stack, and contiguous feeds make them unnecessary.
"""
import sys
sys.path.insert(0, '/opt/trn_rl_repo')

import numpy as np
import concourse.bass as bass
import concourse.bacc as bacc
import concourse.tile as tile
from concourse import mybir

f32 = mybir.dt.float32
bf16 = mybir.dt.bfloat16
u16 = mybir.dt.uint16
Alu = mybir.AluOpType
Act = mybir.ActivationFunctionType

N = 2048
NBLK = N // 128
KNN = 20
NEG_SLOPE = 0.2
BN_EPS = 1e-5
NEG_BIG = -1e30

# (C_in, O) per edge-conv layer
LAYERS = [(3, 64), (64, 64), (64, 128), (128, 256)]
# W5^T host-side chunks aligned to the xcat source tiles
W5_CHUNKS = [(0, 64), (64, 128), (128, 256), (256, 384), (384, 512)]


def _ceil(a, b):
    return (a + b - 1) // b


def build_dgcnn(nc):
    """Emit the full per-core DGCNN program into nc."""
    xT_d = nc.dram_tensor("xT", [3, N], f32, kind="ExternalInput")
    WaT_d = [nc.dram_tensor(f"WaT{l+1}", [c, o], f32, kind="ExternalInput")
             for l, (c, o) in enumerate(LAYERS)]
    WdT_d = [nc.dram_tensor(f"WdT{l+1}", [c, o], f32, kind="ExternalInput")
             for l, (c, o) in enumerate(LAYERS)]
    W5T_d = [nc.dram_tensor(f"W5T_{lo}", [hi - lo, 1024], f32,
                            kind="ExternalInput") for lo, hi in W5_CHUNKS]
    bnT_d = [nc.dram_tensor(f"bnT{l+1}", [o, 4], f32, kind="ExternalInput")
             for l, (c, o) in enumerate(LAYERS)]
    bn5T_d = nc.dram_tensor("bn5T", [1024, 4], f32, kind="ExternalInput")
    out_d = nc.dram_tensor("out", [2048], f32, kind="ExternalOutput")
    # DRAM bounce for the index wrap-relayout, n-major: list[n*20+k] = idx[n,k]
    list_d = nc.dram_tensor("idxlist", [NBLK * 128 * KNN], u16, kind="Internal")

    with tile.TileContext(nc) as tc:
        from contextlib import ExitStack
        ctx = ExitStack()
        with ctx:
            persist = ctx.enter_context(tc.tile_pool(name="persist", bufs=1))
            work = ctx.enter_context(tc.tile_pool(name="work", bufs=2))

            onesC = persist.tile([128, 1], f32, tag="onesC")
            nc.vector.memset(onesC, 1.0)
            ones1 = persist.tile([1, 128], f32, tag="ones1")
            nc.vector.memset(ones1, 1.0)
            eps_t = persist.tile([128, 1], f32, tag="eps")
            nc.vector.memset(eps_t, BN_EPS)

            # input load FIRST: layer 1's prep chain depends on it, and the
            # HWDGE ring is FIFO -- queued behind the ~26 bn/weight loads it
            # would start ~16us late
            x0pool = tc.tile_pool(name="x0pool", bufs=1)
            x0T = x0pool.__enter__().tile([16, N], f32, tag="x0T")
            nc.sync.dma_start(out=x0T[:3, :], in_=xT_d[:, :])

            # ---- bn param prep: (C, 4) rows [gamma, beta, mean, var] ->
            #      scale (C,1), bias (C,1) tiles per 128-channel chunk
            def prep_bn(bn_dram, channels, name):
                scales, biases = [], []
                for t in range(_ceil(channels, 128)):
                    p = min(128, channels - t * 128)
                    raw = work.tile([128, 4], f32, tag="bnraw")
                    src = bass.AP(tensor=bn_dram, offset=t * 128 * 4,
                                  ap=[[4, p], [1, 4]])
                    nc.sync.dma_start(out=raw[:p, :], in_=src)
                    s_t = persist.tile([128, 1], f32, tag=f"{name}_s{t}")
                    b_t = persist.tile([128, 1], f32, tag=f"{name}_b{t}")
                    tmp = work.tile([128, 1], f32, tag="bntmp")
                    nc.scalar.activation(tmp[:p], raw[:p, 3:4], Act.Sqrt,
                                         bias=eps_t[:p], scale=1.0)
                    nc.vector.reciprocal(tmp[:p], tmp[:p])
                    nc.vector.tensor_mul(s_t[:p], raw[:p, 0:1], tmp[:p])
                    nc.vector.tensor_mul(tmp[:p], raw[:p, 2:3], s_t[:p])
                    nc.vector.tensor_sub(b_t[:p], raw[:p, 1:2], tmp[:p])
                    scales.append(s_t)
                    biases.append(b_t)
                return scales, biases

            bn_sb = [prep_bn(bnT_d[l], o, f"bn{l}")
                     for l, (c, o) in enumerate(LAYERS)]
            bn5_s, bn5_b = prep_bn(bn5T_d, 1024, "bn5")

            # ---- weights (already transposed host-side)
            WaT, WdT = [], []
            for l, (c, o) in enumerate(LAYERS):
                wa = persist.tile([max(c, 16), o], f32, tag=f"WaT{l}",
                                  name=f"WaT{l}")
                nc.sync.dma_start(out=wa[:c, :], in_=WaT_d[l][:, :])
                wd = persist.tile([max(c, 16), o], f32, tag=f"WdT{l}",
                                  name=f"WdT{l}")
                nc.sync.dma_start(out=wd[:c, :], in_=WdT_d[l][:, :])
                WaT.append(wa)
                WdT.append(wd)
            W5T = []
            for i, (lo, hi) in enumerate(W5_CHUNKS):
                # gpsimd (SWDGE) cast DMA f32->bf16, bit-matching the
                # numerics of the passing baseline final layer
                t5 = persist.tile([max(hi - lo, 16), 1024], bf16,
                                  tag=f"W5T_{lo}", name=f"W5T_{lo}")
                nc.gpsimd.dma_start(out=t5[:hi - lo, :], in_=W5T_d[i][:, :])
                W5T.append(t5)

            # ---- edge conv layers
            def edge_conv(l, c, o, xT, out_tag):
                notile = _ceil(o, 128)
                aug = c + 1 <= 65
                with ExitStack() as lx:
                    lwork = lx.enter_context(
                        tc.tile_pool(name=f"lwork{l}", bufs=2))
                    prep_ps = tc.tile_pool(name=f"prep_ps{l}", bufs=2,
                                           space="PSUM")
                    with prep_ps as pp:
                        sq = lwork.tile([max(c, 16), N], f32, tag="sq", bufs=1)
                        nc.scalar.square(sq[:c, :], xT[:c, :])
                        if aug:
                            lhs_sc = lwork.tile([c + 1, N], f32, tag="lhs_sc",
                                                bufs=1)
                            rhs_sc = lwork.tile([c + 1, N], f32, tag="rhs_sc",
                                                bufs=1)
                            aligned = (c % 32) == 0
                            if aligned:
                                nc.vector.tensor_scalar_mul(lhs_sc[:c, :],
                                                            xT[:c, :], 2.0)
                                nc.vector.memset(lhs_sc[c:c + 1, :], 1.0)
                                nc.scalar.copy(rhs_sc[:c, :], xT[:c, :])
                                negsq_dst = rhs_sc[c:c + 1, :]
                            else:
                                # engine APs must start 32-aligned: fill the
                                # ones row via full-height memset; negsq goes
                                # through a base-0 tile + contiguous SBUF DMA
                                nc.vector.memset(lhs_sc[:c + 1, :], 1.0)
                                nc.vector.tensor_scalar_mul(lhs_sc[:c, :],
                                                            xT[:c, :], 2.0)
                                nc.scalar.copy(rhs_sc[:c, :], xT[:c, :])
                                negsq = lwork.tile([1, N], f32, tag="negsq",
                                                   bufs=1)
                                negsq_dst = negsq[:, :]
                        else:
                            lhs_sc = lwork.tile([c, N], f32, tag="lhs_sc",
                                                bufs=1)
                            rhs_sc = xT
                            nc.vector.tensor_scalar_mul(lhs_sc[:c, :],
                                                        xT[:c, :], 2.0)
                            negsq = lwork.tile([1, N], f32, tag="negsq", bufs=1)
                            negsq_dst = negsq[:, :]
                        for ch in range(4):
                            cs = slice(ch * 512, (ch + 1) * 512)
                            nps = pp.tile([1, 512], f32, tag="negsq_ps")
                            nc.tensor.matmul(nps, lhsT=onesC[:c, :],
                                             rhs=sq[:c, cs],
                                             start=True, stop=True)
                            nc.scalar.mul(negsq_dst[:, cs], nps, -1.0)
                        if aug and not aligned:
                            nc.sync.dma_start(out=rhs_sc[c:c + 1, :],
                                              in_=negsq[:, :])

                    # --- u, v tiles; matmul units are emitted inside the
                    # block loop (all u at block 0 BEFORE the first gather
                    # emission, v at blocks 1-2 BEFORE the first tail) so the
                    # layer's first score matmuls aren't queued behind them on
                    # the PE. Program-order rule: every u_sb/v_sb write is
                    # emitted before its first reader.
                    uv_ps = lx.enter_context(
                        tc.tile_pool(name=f"uv_ps{l}", bufs=2, space="PSUM"))
                    u_sb = [lwork.tile([128, N], f32, tag=f"u_sb{t}",
                                       name=f"u_sb{t}", bufs=1)
                            for t in range(notile)]
                    v_sb = [lwork.tile([128, N], f32, tag=f"v_sb{t}",
                                       name=f"v_sb{t}", bufs=1)
                            for t in range(notile)]
                    rem = o - (notile - 1) * 128
                    if rem < 128:
                        nc.vector.memset(u_sb[notile - 1][rem:, :], 0.0)

                    def emit_uv(which, chunks):
                        for t in range(notile):
                            op = min(128, o - t * 128)
                            osl = slice(t * 128, t * 128 + op)
                            for ch in chunks:
                                cs = slice(ch * 512, (ch + 1) * 512)
                                if which == 'u':
                                    ups = uv_ps.tile([128, 512], f32,
                                                     tag="u_ps")
                                    nc.tensor.matmul(ups[:op, :],
                                                     lhsT=WaT[l][:c, osl],
                                                     rhs=xT[:c, cs],
                                                     start=True, stop=True)
                                    nc.scalar.copy(u_sb[t][:op, cs],
                                                   ups[:op, :])
                                else:
                                    vps = uv_ps.tile([128, 512], f32,
                                                     tag="v_ps")
                                    nc.tensor.matmul(vps[:op, :],
                                                     lhsT=WdT[l][:c, osl],
                                                     rhs=xT[:c, cs],
                                                     start=True, stop=True)
                                    nc.scalar.copy(v_sb[t][:op, cs],
                                                   vps[:op, :])

                    # --- block loop, software-pipelined: the per-block tail
                    # (gather -> k-max -> +v -> bn -> lrelu) is emitted LAG
                    # blocks behind its top-k, so the DVE top-k stream never
                    # stalls on the Pool-engine gathers, and Pool gathers for
                    # block b run while the DVE does top-k of block b+LAG.
                    # All fp32 tail math is bit-exact on any engine; the add
                    # and leaky-relu ride GPSIMD to keep the DVE lean.
                    xout = [persist.tile([128, N], f32, tag=f"{out_tag}_{t}",
                                         name=f"{out_tag}_{t}")
                            for t in range(notile)]
                    wrap_t = [lwork.tile([128, 160], u16, tag=f"wrap{b}",
                                         name=f"wrap{b}", bufs=1)
                              for b in range(NBLK)]
                    # Tail lag (blocks) behind the top-k: must cover the
                    # idx-DMA-bounce + gather latency (~22us) so the reduce is
                    # always ready when the static schedule reaches it. L4
                    # (notile=2) is SBUF-tight: shallower buffering there.
                    LAG = 10 if notile == 1 else 5
                    gath_bufs = 7
                    sc_bufs = 2

                    def block_gather(b):
                        # Pool-engine gathers, emitted right behind block b's
                        # top-k so the Pool queue always has gathers ahead of
                        # the cross-engine tail round-trips.
                        tiles = []
                        for t in range(notile):
                            gath = lwork.tile([128, 2560], f32, tag="gath",
                                              bufs=gath_bufs)
                            # walrus caps indirect_copy at 1024 indices
                            for lo in range(0, 2560, 1024):
                                hi = min(lo + 1024, 2560)
                                nc.gpsimd.indirect_copy(
                                    gath[:, lo:hi], u_sb[t],
                                    wrap_t[b][:, lo // 16:hi // 16], True)
                            tiles.append(gath)
                        return tiles

                    def block_tail(b, gtiles):
                        bsl = slice(b * 128, (b + 1) * 128)
                        for t in range(notile):
                            op = min(128, o - t * 128)
                            m_sb = lwork.tile([128, 128], f32, tag="m_sb")
                            nc.vector.tensor_reduce(
                                m_sb,
                                gtiles[t].rearrange("p (n k) -> p n k", k=KNN),
                                axis=mybir.AxisListType.X, op=Alu.max)
                            y = lwork.tile([128, 128], f32, tag="yb")
                            nc.vector.tensor_add(y[:op, :], m_sb[:op, :],
                                                 v_sb[t][:op, bsl])
                            ybn = lwork.tile([128, 128], f32, tag="ybn")
                            nc.scalar.activation(ybn[:op, :], y[:op, :],
                                                 Act.Identity,
                                                 bias=bn_sb[l][1][t][:op],
                                                 scale=bn_sb[l][0][t][:op])
                            nc.vector.scalar_tensor_tensor(
                                xout[t][:op, bsl], ybn[:op, :], NEG_SLOPE,
                                ybn[:op, :], op0=Alu.mult, op1=Alu.max)

                    pending = {}
                    with tc.tile_pool(name=f"sc_ps{l}", bufs=4,
                                      space="PSUM") as sp:
                        for b in range(NBLK):
                            bsl = slice(b * 128, (b + 1) * 128)
                            # scores per 1-bank PSUM chunk, staged to SBUF by
                            # exact f32 copies: each bank frees early so the
                            # PE streams ahead at full clock instead of
                            # idling into a low p-state
                            sc = lwork.tile([128, N], f32, tag="sc",
                                            bufs=sc_bufs)
                            for ch in range(4):
                                cs = slice(ch * 512, (ch + 1) * 512)
                                scps = sp.tile([128, 512], f32, tag="scps")
                                if aug:
                                    nc.tensor.matmul(scps,
                                                     lhsT=lhs_sc[:c + 1, bsl],
                                                     rhs=rhs_sc[:c + 1, cs],
                                                     start=True, stop=True)
                                else:
                                    nc.tensor.matmul(scps,
                                                     lhsT=lhs_sc[:c, bsl],
                                                     rhs=rhs_sc[:c, cs],
                                                     start=True, stop=False)
                                    nc.tensor.matmul(scps, lhsT=ones1,
                                                     rhs=negsq[:, cs],
                                                     start=False, stop=True)
                                nc.scalar.copy(sc[:, cs], scps)
                            idxb = lwork.tile([128, 24], u16, tag="idxb")
                            vals = lwork.tile([128, 8], f32, tag="vals")
                            nc.vector.max(vals, sc)
                            nc.vector.max_index(idxb[:, 0:8], vals, sc)
                            nc.vector.match_replace(sc, vals, sc, NEG_BIG)
                            nc.vector.max(vals, sc)
                            nc.vector.max_index(idxb[:, 8:16], vals, sc)
                            nc.vector.match_replace(sc, vals, sc, NEG_BIG)
                            nc.vector.max(vals, sc)
                            nc.vector.max_index(idxb[:, 16:24], vals, sc)

                            # n-major contiguous store: list[n*20+k] = idxb[n,k]
                            dst1 = bass.AP(tensor=list_d, offset=b * 2560,
                                           ap=[[KNN, 128], [1, KNN]])
                            nc.sync.dma_start(out=dst1, in_=idxb[:, 0:KNN])
                            # wrap read: wrapped[16g+p, s] = list[s*16+p]
                            for g in range(8):
                                src2 = bass.AP(tensor=list_d, offset=b * 2560,
                                               ap=[[1, 16], [16, 160]])
                                nc.sync.dma_start(
                                    out=wrap_t[b][g * 16:(g + 1) * 16, :],
                                    in_=src2)
                            # u/v interleave: all u before the first gather
                            # emission (gathers read u_sb), v split over
                            # blocks 1-2 (first tail reads v at b=LAG>=2)
                            if b == 0:
                                emit_uv('u', range(4))
                            elif b == 1:
                                emit_uv('v', (0, 1))
                            elif b == 2:
                                emit_uv('v', (2, 3))
                            pending[b] = block_gather(b)
                            if b >= LAG:
                                block_tail(b - LAG, pending.pop(b - LAG))
                        for b in range(NBLK - LAG, NBLK):
                            block_tail(b, pending.pop(b))
                return xout

            x1 = edge_conv(0, 3, 64, x0T, "x1")
            x0pool.__exit__(None, None, None)
            x2 = edge_conv(1, 64, 64, x1[0], "x2")
            x3 = edge_conv(2, 64, 128, x2[0], "x3")
            x4 = edge_conv(3, 128, 256, x3[0], "x4")

            # ---- final 1x1 conv (W5, f32) + BN + lrelu + global max/avg pool
            xcat_parts = [(x1[0], 64), (x2[0], 64), (x3[0], 128),
                          (x4[0], 128), (x4[1], 128)]
            inv_n = 1.0 / float(N)
            with tc.tile_pool(name="f_ps", bufs=4, space="PSUM") as fp, \
                 tc.tile_pool(name="fwork", bufs=2) as fw, \
                 tc.tile_pool(name="fb16", bufs=1) as fb:
                # bf16 staging + Identity/stt tail: numerically identical to
                # the passing baseline final layer
                xcb = []
                for i, (xp, ck) in enumerate(xcat_parts):
                    xtile = fb.tile([max(ck, 16), N], bf16, tag=f"xcb{i}",
                                    name=f"xcb{i}")
                    nc.vector.tensor_copy(xtile[:ck, :], xp[:ck, :])
                    xcb.append(xtile)
                for ot in range(8):
                    osl = slice(ot * 128, (ot + 1) * 128)
                    sums = fw.tile([128, 4], f32, tag="sums")
                    gmax = fw.tile([128, 512], f32, tag="gmax512")
                    for chn in range(4):
                        cs = slice(chn * 512, (chn + 1) * 512)
                        fps = fp.tile([128, 512], f32, tag="fps")
                        for i, (xp, ck) in enumerate(xcat_parts):
                            nc.tensor.matmul(fps, lhsT=W5T[i][:ck, osl],
                                             rhs=xcb[i][:ck, cs],
                                             start=(i == 0), stop=(i == 4))
                        ybn = fw.tile([128, 512], f32, tag="fybn")
                        nc.scalar.activation(ybn, fps, Act.Identity,
                                             bias=bn5_b[ot], scale=bn5_s[ot])
                        feat = fw.tile([128, 512], f32, tag="feat")
                        nc.vector.scalar_tensor_tensor(
                            feat, ybn, NEG_SLOPE, ybn,
                            op0=Alu.mult, op1=Alu.max,
                            accum_out=sums[:, chn:chn + 1])
                        if chn == 0:
                            nc.vector.tensor_copy(gmax, feat)
                        else:
                            nc.vector.tensor_max(gmax, gmax, feat)
                    gm = fw.tile([128, 1], f32, tag="gm")
                    nc.vector.tensor_reduce(gm, gmax, axis=mybir.AxisListType.X,
                                            op=Alu.max)
                    ga = fw.tile([128, 1], f32, tag="ga")
                    nc.vector.tensor_reduce(ga, sums, axis=mybir.AxisListType.X,
                                            op=Alu.add)
                    nc.vector.tensor_scalar_mul(ga, ga, inv_n)
                    nc.sync.dma_start(
                        out=bass.AP(tensor=out_d, offset=ot * 128,
                                    ap=[[1, 128]]),
                        in_=gm[:, :])
                    nc.sync.dma_start(
                        out=bass.AP(tensor=out_d, offset=1024 + ot * 128,
                                    ap=[[1, 128]]),
                        in_=ga[:, :])


def host_prepare(inputs):
    """Full inputs -> per-core input maps (host-side layout transforms)."""
    x = np.asarray(inputs["x"], dtype=np.float32)
    B = x.shape[0]
    shared = {}
    for l, (c, o) in enumerate(LAYERS):
        W = np.asarray(inputs[f"W{l+1}"], dtype=np.float32)
        Wa = W[:, :c]
        Wd = W[:, c:] - Wa
        shared[f"WaT{l+1}"] = np.ascontiguousarray(Wa.T)
        shared[f"WdT{l+1}"] = np.ascontiguousarray(Wd.T)
        bn = np.asarray(inputs[f"bn{l+1}"], dtype=np.float32)
        shared[f"bnT{l+1}"] = np.ascontiguousarray(bn.T)
    W5 = np.asarray(inputs["W5"], dtype=np.float32)
    for lo, hi in W5_CHUNKS:
        shared[f"W5T_{lo}"] = np.ascontiguousarray(W5[:, lo:hi].T)
    shared["bn5T"] = np.ascontiguousarray(
        np.asarray(inputs["bn5"], dtype=np.float32).T)
    return [dict(shared, xT=np.ascontiguousarray(x[b].T)) for b in range(B)]


_CACHED = {}


def _get_compiled():
    if "nc" not in _CACHED:
        nc = bacc.Bacc("TRN2", target_bir_lowering=False, debug=False,
                       num_devices=8)
        build_dgcnn(nc)
        nc.compile()
        _CACHED["nc"] = nc
    return _CACHED["nc"]


def kernel(**inputs):
    from concourse.bass_utils import run_bass_kernel_spmd
    nc = _get_compiled()
    in_maps = host_prepare(inputs)
    res = run_bass_kernel_spmd(nc, in_maps, list(range(len(in_maps))))
    out = np.stack([np.asarray(res.results[b]["out"]).reshape(-1)
                    for b in range(len(in_maps))], axis=0)
    return out.astype(np.float32)
